# revision 1
# baseline (speedup 1.0000x reference)
"""HeteroMoE layer for Trainium2, 8-core SPMD.

Routing is top-1 with weight exactly 1.0, so out[b] = expert_{argmax(logits[b])}(x[b]).
Host computes routing (tiny), permutes the batch into 8 cores x 4 slots so that
each slot has a uniform compile-time "mode" (which dw-conv dilation taps / whether
the expert-2 1x1 pre-matmul is needed). All per-element selection is carried in
per-core parameter tensors (BN scale folded into weights); a zero weight makes an
unused op a no-op. Depthwise 3x3 convs run as 9 diagonal fp16 matmuls on the PE
accumulating in PSUM (together with the optional expert-2 1x1), gelu+BN-bias on
the scalar engine, the pointwise 1x1 as fp16 matmuls, bias-add on the scalar
engine (Identity + per-partition bias), all accumulating in fp32 PSUM.
"""
import numpy as np

import concourse.bacc as bacc
import concourse.tile as tile
import concourse.mybir as mybir
from concourse.bass_utils import run_bass_kernel_spmd

F32 = mybir.dt.float32
F16 = mybir.dt.float16

B, C, H, W = 32, 256, 64, 64
HW = H * W
NCORES = 8
NSLOT = B // NCORES
PAD = 2
R = W + 2 * PAD          # padded row stride (68)
TPAD = 3                 # top/bottom pad rows (extra margin for 1-D DVE runs)
RH = H + 2 * TPAD        # padded rows (70)
PADHW = R * RH
ACC0 = TPAD * R + PAD    # 1-D run start: first interior position
ACCL = (H - 1) * R + W   # 1-D run length (63*68+64 = 4348)
NBLK = C // 128          # 2 channel blocks
EPS = 1e-5

_CACHE = {}


def _offsets(dil):
    return [(dy * dil, dx * dil) for dy in (-1, 0, 1) for dx in (-1, 0, 1)]


def _slot_offsets(mode):
    if mode == "d1":
        return _offsets(1)
    if mode == "d2":
        return _offsets(2)
    if mode == "d12":
        s = _offsets(1) + [o for o in _offsets(2) if o != (0, 0)]
        return s
    return []


def build(slot_modes, tap_eng, repeat=1):
    """slot_modes: (tapmode, has_m) per slot; tap_eng: "pe"|"dve" per slot"""
    nc = bacc.Bacc("TRN2", target_bir_lowering=False, debug=False,
                   num_devices=NCORES)
    xin = nc.dram_tensor("xin", [NSLOT, C, HW], F32, kind="ExternalInput").ap()
    yout = nc.dram_tensor("yout", [NSLOT, C, HW], F32, kind="ExternalOutput").ap()
    prm = {}
    for s, (tm, has_m) in enumerate(slot_modes):
        offs = _slot_offsets(tm)
        if offs:
            if tap_eng[s] == "dve":
                prm[f"dk{s}"] = nc.dram_tensor(
                    f"dk{s}", [128, NBLK * len(offs)], F32,
                    kind="ExternalInput").ap()
            else:
                prm[f"dk{s}"] = nc.dram_tensor(
                    f"dk{s}", [128, NBLK * len(offs) * 128], F16,
                    kind="ExternalInput").ap()
        if has_m:
            prm[f"mw{s}"] = nc.dram_tensor(
                f"mw{s}", [128, NBLK * NBLK * 128], F16, kind="ExternalInput").ap()
        prm[f"pw{s}"] = nc.dram_tensor(
            f"pw{s}", [128, NBLK * NBLK * 128], F16, kind="ExternalInput").ap()
        prm[f"tb{s}"] = nc.dram_tensor(f"tb{s}", [128, NBLK], F32,
                                       kind="ExternalInput").ap()
        prm[f"qb{s}"] = nc.dram_tensor(f"qb{s}", [128, NBLK], F32,
                                       kind="ExternalInput").ap()

    with tile.TileContext(nc) as tc:
        with tc.tile_pool(name="params", bufs=1) as ppool, \
             tc.tile_pool(name="xplain", bufs=6) as xpool, \
             tc.tile_pool(name="x16", bufs=1) as cpool, \
             tc.tile_pool(name="a16", bufs=2) as apool, \
             tc.tile_pool(name="zacc", bufs=1) as zpool, \
             tc.tile_pool(name="x16o", bufs=1) as oppool, \
             tc.tile_pool(name="o32", bufs=4) as opool, \
             tc.tile_pool(name="psz", bufs=2, space="PSUM") as pszp, \
             tc.tile_pool(name="psw", bufs=4, space="PSUM") as pswp:

            # resident params
            pt = {}
            for name, ap in prm.items():
                t = ppool.tile(list(ap.shape), ap.dtype, tag=name, name=name)
                nc.sync.dma_start(t[:], ap)
                pt[name] = t

            for rep in range(repeat):
                for s, (tm, has_m) in enumerate(slot_modes):
                    offs = _slot_offsets(tm)
                    ntap = len(offs)
                    # --- load + convert to fp16 ---
                    # padded fp16 tiles (also used unpadded-interior for
                    # matmul-only slots); borders must read as zero
                    x16 = [cpool.tile([128, PADHW], F16, tag=f"x16p{_b}",
                                      name=f"x16p{_b}")
                           for _b in range(NBLK)]
                    for bk in range(NBLK):
                        if ntap:
                            nc.gpsimd.memset(x16[bk][:], 0)
                        x3 = x16[bk][:].rearrange("p (h w) -> p h w", h=RH, w=R)
                        for hf in range(2):
                            xp = xpool.tile([128, HW // 2], F32, tag="xp",
                                            name="xp")
                            nc.sync.dma_start(
                                xp[:], xin[s, bk * 128:(bk + 1) * 128,
                                           hf * (HW // 2):(hf + 1) * (HW // 2)])
                            dst = x3[:, TPAD + hf * (H // 2):
                                     TPAD + (hf + 1) * (H // 2),
                                     PAD:PAD + W]
                            src = xp[:].rearrange("p (h w) -> p h w",
                                                  h=H // 2, w=W)
                            nc.vector.tensor_copy(dst, src)

                    def rwin(bk, chunk, dy, dx):
                        ap3 = x16[bk][:].rearrange(
                            "p (h w) -> p h w", h=RH, w=R)
                        r0 = TPAD + dy + chunk * 8
                        c0 = PAD + dx
                        return ap3[:, r0:r0 + 8, c0:c0 + W]

                    # --- stage 1: z = taps + optional M@x ; gelu -> a16 ---
                    a16 = [apool.tile([128, HW], F16, tag=f"a16{_b}", name=f"a16{_b}")
                           for _b in range(NBLK)]
                    if ntap and tap_eng[s] == "dve":
                        # fp16 STT tap chain on the vector engine, 1-D runs
                        # over padded coords; odd offsets read a shift-by-one
                        # copy so the 2x packed mode always engages
                        offs1d = [dy * R + dx for (dy, dx) in offs]
                        need_odd = any(o % 2 for o in offs1d)
                        if need_odd:
                            x16o = [oppool.tile([128, PADHW], F16,
                                                tag=f"x16o{_b}", name=f"x16o{_b}")
                                    for _b in range(NBLK)]
                            for bk in range(NBLK):
                                nc.vector.tensor_copy(
                                    x16o[bk][:, 0:PADHW - 1],
                                    x16[bk][:, 1:PADHW])
                        for cb in range(NBLK):
                            acc = zpool.tile([128, PADHW], F16,
                                             tag=f"zacc{cb}", name=f"zacc{cb}")
                            for t, o in enumerate(offs1d):
                                if o % 2:
                                    src = x16o[cb][:, ACC0 + o - 1:
                                                   ACC0 + o - 1 + ACCL]
                                else:
                                    src = x16[cb][:, ACC0 + o:ACC0 + o + ACCL]
                                dkcol = pt[f"dk{s}"][:, cb * ntap + t:
                                                     cb * ntap + t + 1]
                                if t == 0:
                                    nc.vector.tensor_scalar_mul(
                                        acc[:, ACC0:ACC0 + ACCL], src, dkcol)
                                else:
                                    nc.vector.scalar_tensor_tensor(
                                        acc[:, ACC0:ACC0 + ACCL], src, dkcol,
                                        acc[:, ACC0:ACC0 + ACCL],
                                        op0=mybir.AluOpType.mult,
                                        op1=mybir.AluOpType.add)
                            zin = acc[:].rearrange(
                                "p (h w) -> p h w", h=RH, w=R)[
                                :, TPAD:TPAD + H, PAD:PAD + W]
                            nc.scalar.activation(
                                a16[cb][:].rearrange("p (h w) -> p h w",
                                                     h=H, w=W),
                                zin,
                                mybir.ActivationFunctionType.Gelu,
                                bias=pt[f"tb{s}"][:, cb:cb + 1], scale=1.0)
                        stage1_pe = False
                    else:
                        stage1_pe = True
                    for cb in (range(NBLK) if stage1_pe else []):
                        for hf4 in range(4):
                            psz = pszp.tile([128, 1024], F32, tag="psz", name="psz")
                            for q in range(2):
                                chunk = hf4 * 2 + q
                                pslice = psz[:, q * 512:(q + 1) * 512]
                                first = True
                                for t, (dy, dx) in enumerate(offs):
                                    lhsT = pt[f"dk{s}"][
                                        :, (cb * ntap + t) * 128:
                                           (cb * ntap + t + 1) * 128]
                                    nc.tensor.matmul(
                                        pslice, lhsT, rwin(cb, chunk, dy, dx),
                                        start=first,
                                        stop=(not has_m and t == ntap - 1))
                                    first = False
                                if has_m:
                                    for ib in range(NBLK):
                                        lhsT = pt[f"mw{s}"][
                                            :, (ib * NBLK + cb) * 128:
                                               (ib * NBLK + cb + 1) * 128]
                                        nc.tensor.matmul(
                                            pslice, lhsT, rwin(ib, chunk, 0, 0),
                                            start=first,
                                            stop=(ib == NBLK - 1))
                                        first = False
                            nc.scalar.activation(
                                a16[cb][:, hf4 * 1024:(hf4 + 1) * 1024],
                                psz[:],
                                mybir.ActivationFunctionType.Gelu,
                                bias=pt[f"tb{s}"][:, cb:cb + 1], scale=1.0)

                    # --- stage 2: pointwise + bias, streamed out in halves ---
                    for cb in range(NBLK):
                        for half in range(2):
                            o32 = opool.tile([128, HW // 2], F32, tag="o32",
                                             name="o32")
                            for q in range(4):
                                chunk = half * 4 + q
                                psw = pswp.tile([128, 512], F32, tag="psw",
                                                name="psw")
                                for ib in range(NBLK):
                                    lhsT = pt[f"pw{s}"][
                                        :, (ib * NBLK + cb) * 128:
                                           (ib * NBLK + cb + 1) * 128]
                                    nc.tensor.matmul(
                                        psw[:], lhsT,
                                        a16[ib][:, chunk * 512:(chunk + 1) * 512],
                                        start=(ib == 0), stop=(ib == NBLK - 1))
                                nc.scalar.activation(
                                    o32[:, q * 512:(q + 1) * 512], psw[:],
                                    mybir.ActivationFunctionType.Identity,
                                    bias=pt[f"qb{s}"][:, cb:cb + 1], scale=1.0)
                            nc.sync.dma_start(
                                yout[s, cb * 128:(cb + 1) * 128,
                                     half * (HW // 2):(half + 1) * (HW // 2)],
                                o32[:])
    nc.compile()
    return nc


def _plan(idx):
    """Assign elements to (core, slot); return slot_modes, elem[core][slot]."""
    by = [list(np.where(idx == t)[0]) for t in range(3)]
    n0, n1, n2 = map(len, by)
    groups = []  # (mode, has_m, [elems])
    for t, mode in ((0, "d1"), (1, "d2")):
        while len(by[t]) >= 8:
            groups.append([mode, False, by[t][:8]])
            by[t] = by[t][8:]
    # remainders share slots with e2 padding
    for t, mode in ((0, "d1"), (1, "d2")):
        if by[t]:
            take = min(8 - len(by[t]), len(by[2]))
            g = by[t] + by[2][:take]
            by[2] = by[2][take:]
            by[t] = []
            groups.append([mode, take > 0, g])
    while by[2]:
        groups.append([None, True, by[2][:8]])
        by[2] = by[2][8:]
    # merge if >4 groups (rare): combine two tap groups into d12
    while len(groups) > NSLOT:
        tapg = [g for g in groups if g[0] is not None]
        a, b = tapg[-2], tapg[-1]
        groups.remove(b)
        a[0] = "d12"
        a[1] = a[1] or b[1]
        a[2] += b[2]
        assert len(a[2]) <= 8
    # pad groups to exactly 8 elems (reuse element 0 as dummy -> wasted compute,
    # result discarded) and to exactly NSLOT groups
    for g in groups:
        g.append(len(g[2]))
        while len(g[2]) < 8:
            g[2].append(-1)
    while len(groups) < NSLOT:
        groups.append([None, False, [-1] * 8, 0])
    slot_modes = tuple((g[0], g[1]) for g in groups)
    elem = [[groups[s][2][c] for s in range(NSLOT)] for c in range(NCORES)]
    return slot_modes, elem


def _fold_params(kw):
    """Per expert: BN-folded weights. Returns dicts."""
    out = {}
    for i in range(3):
        g = kw[f"e{i}_g"]; b = kw[f"e{i}_b"]; m = kw[f"e{i}_m"]; v = kw[f"e{i}_v"]
        s = g / np.sqrt(v + EPS)
        t = b - m * s
        out[i] = dict(s=s.astype(np.float32), t=t.astype(np.float32),
                      pw=kw[f"e{i}_pw"].astype(np.float32),
                      pb=kw[f"e{i}_pb"].astype(np.float32))
        if i < 2:
            out[i]["k"] = (kw[f"e{i}_k"].reshape(C, 9) * s[:, None]).astype(np.float32)
        else:
            out[i]["M"] = (kw["e2_k"] * s[:, None]).astype(np.float32)
    return out


def _make_inmaps(x, idx, elem, slot_modes, tap_eng, fold):
    in_maps = []
    d1off = _slot_offsets("d1")
    for c in range(NCORES):
        im = {}
        xs = np.zeros((NSLOT, C, HW), np.float32)
        for s in range(NSLOT):
            e = elem[c][s]
            if e >= 0:
                xs[s] = x[e].reshape(C, HW)
        im["xin"] = xs
        for s, (tm, has_m) in enumerate(slot_modes):
            offs = _slot_offsets(tm)
            ntap = len(offs)
            e = elem[c][s]
            t_e = idx[e] if e >= 0 else -1
            f = fold[t_e] if t_e >= 0 else None
            if ntap:
                if tap_eng[s] == "dve":
                    dk = np.zeros((128, NBLK * ntap), np.float32)
                    if f is not None and t_e < 2:
                        myoffs = _slot_offsets("d1" if t_e == 0 else "d2")
                        for ti, o in enumerate(offs):
                            if o in myoffs:
                                ki = myoffs.index(o)
                                for bk in range(NBLK):
                                    dk[:, bk * ntap + ti] = \
                                        f["k"][bk * 128:(bk + 1) * 128, ki]
                else:
                    dk = np.zeros((128, NBLK * ntap * 128), np.float16)
                    if f is not None and t_e < 2:
                        myoffs = _slot_offsets("d1" if t_e == 0 else "d2")
                        for ti, o in enumerate(offs):
                            if o in myoffs:
                                ki = myoffs.index(o)
                                for bk in range(NBLK):
                                    col = (bk * ntap + ti) * 128
                                    dk[:, col:col + 128][np.arange(128), np.arange(128)] = \
                                        f["k"][bk * 128:(bk + 1) * 128, ki].astype(np.float16)
                im[f"dk{s}"] = dk
            if has_m:
                mw = np.zeros((128, NBLK * NBLK * 128), np.float16)
                if f is not None and t_e == 2:
                    M = f["M"]
                    for ib in range(NBLK):
                        for cb in range(NBLK):
                            blk = M[cb * 128:(cb + 1) * 128,
                                    ib * 128:(ib + 1) * 128].T
                            col = (ib * NBLK + cb) * 128
                            mw[:, col:col + 128] = blk.astype(np.float16)
                im[f"mw{s}"] = mw
            pw = np.zeros((128, NBLK * NBLK * 128), np.float16)
            tb = np.zeros((128, NBLK), np.float32)
            qb = np.zeros((128, NBLK), np.float32)
            if f is not None:
                P = f["pw"]
                for ib in range(NBLK):
                    for cb in range(NBLK):
                        blk = P[cb * 128:(cb + 1) * 128,
                                ib * 128:(ib + 1) * 128].T
                        col = (ib * NBLK + cb) * 128
                        pw[:, col:col + 128] = blk.astype(np.float16)
                tb[:] = f["t"].reshape(NBLK, 128).T
                qb[:] = f["pb"].reshape(NBLK, 128).T
            im[f"pw{s}"] = pw
            im[f"tb{s}"] = tb
            im[f"qb{s}"] = qb
        in_maps.append(im)
    return in_maps


def kernel(**inputs):
    inputs = {k: np.asarray(v) for k, v in inputs.items()}
    x = np.ascontiguousarray(inputs["x"], np.float32)
    rw = np.asarray(inputs["rw"], np.float32)
    rb = np.asarray(inputs["rb"], np.float32)
    pooled = x.mean(axis=(2, 3), dtype=np.float32)
    logits = pooled @ rw.T + rb
    idx = logits.argmax(-1)

    slot_modes, elem = _plan(idx)
    tap_eng = tuple("pe" for _ in slot_modes)
    fold = _fold_params(inputs)
    in_maps = _make_inmaps(x, idx, elem, slot_modes, tap_eng, fold)

    key = (slot_modes, tap_eng)
    if key not in _CACHE:
        _CACHE[key] = build(slot_modes, tap_eng)
    nc = _CACHE[key]
    res = run_bass_kernel_spmd(nc, in_maps, core_ids=list(range(NCORES)),
                               trace=False)
    out = np.zeros((B, C, H, W), np.float32)
    for c in range(NCORES):
        yo = res.results[c]["yout"]
        for s in range(NSLOT):
            e = elem[c][s]
            if e >= 0:
                out[e] = yo[s].reshape(C, H, W)
    return out



# revision 14
# speedup vs baseline: 1.8734x; 1.8734x over previous
"""HeteroMoE layer for Trainium2, 8-core SPMD.

Top-1 routing with weight 1.0: out[b] = expert_{argmax(logits[b])}(x[b]).
Host computes routing and permutes the batch into 8 cores x 4 slots with a
uniform compile-time mode per slot. Depthwise 3x3 convs run as fp8e4m3
DoubleRow matmuls on the PE: each DR matmul applies TWO taps at once via a
diagonal-pair stationary operand and an overlapping-stride moving AP (the
k-subtile dim strides by the offset difference between the paired taps).
The expert-2 1x1 pre-matmul also runs as one fp8 DR matmul (K=256 via two
channel-block planes). BN is folded into the conv weights; gelu+BN-bias on
the scalar engine; the final pointwise 1x1 runs in fp16; PSUM->SBUF
copy+bias is spread across DVE/Pool/Act; fp16 output.
"""
import numpy as np
import ml_dtypes

import concourse.bacc as bacc
import concourse.tile as tile
import concourse.mybir as mybir
from concourse.ap import AP
from concourse.bass_utils import run_bass_kernel_spmd

F32 = mybir.dt.float32
F16 = mybir.dt.float16
F8 = mybir.dt.float8e4
NPF8 = ml_dtypes.float8_e4m3
DRMODE = mybir.MatmulPerfMode.DoubleRow

B, C, H, W = 32, 256, 64, 64
HW = H * W
NCORES = 8
NSLOT = B // NCORES
PAD = 2                  # left/right pad cols
TPAD = 2                 # top pad rows
BPAD = 3                 # bottom pad rows (+1 so the dummy-tap pair stays
                         # in bounds at the last chunk)
R = W + 2 * PAD          # padded row stride (68)
RH = H + TPAD + BPAD     # padded rows (69)
PADHW = R * RH           # 4692
NBLK = C // 128          # 2 channel blocks
EPS = 1e-5

_CACHE = {}


def _offsets(dil):
    return [(dy * dil, dx * dil) for dy in (-1, 0, 1) for dx in (-1, 0, 1)]


def _slot_offsets(mode):
    if mode == "d1":
        return _offsets(1)
    if mode == "d2":
        return _offsets(2)
    if mode == "d12":
        return _offsets(1) + [o for o in _offsets(2) if o != (0, 0)]
    return []


def _offs1d(mode):
    """Sorted 1-D offsets for the slot mode, plus per-pair layout.

    Returns (offs, pairs) where offs is the sorted offset list and pairs is a
    list of (o_first, delta, [plane0_off, plane1_off|None]); a None plane means
    zero weights (dummy tap reading in-bounds garbage at o_first+1).
    """
    offs = sorted(dy * R + dx for (dy, dx) in _slot_offsets(mode))
    pairs = []
    i = 0
    while i < len(offs):
        if i + 1 < len(offs):
            pairs.append((offs[i], offs[i + 1] - offs[i], (offs[i], offs[i + 1])))
            i += 2
        else:
            pairs.append((offs[i], 1, (offs[i], None)))
            i += 1
    return offs, pairs


def build(slot_modes, copy_eng, repeat=1):
    """slot_modes: (tapmode, has_m) per slot; copy_eng: per-slot engine for the
    stage-2 PSUM->SBUF copy ("dve"|"pool"|"act")."""
    nc = bacc.Bacc("TRN2", target_bir_lowering=False, debug=False,
                   num_devices=NCORES)
    xin = nc.dram_tensor("xin", [NSLOT, C, PADHW], F8, kind="ExternalInput").ap()
    m_slots = [s for s, (tm, hm) in enumerate(slot_modes) if hm]
    xm = {s: nc.dram_tensor(f"xm{s}", [C, HW], F16, kind="ExternalInput").ap()
          for s in m_slots}
    yout = nc.dram_tensor("yout", [NSLOT, C, HW], F16, kind="ExternalOutput").ap()
    prm = {}
    for s, (tm, has_m) in enumerate(slot_modes):
        offs, pairs = _offs1d(tm)
        if offs:
            prm[f"dk{s}"] = nc.dram_tensor(
                f"dk{s}", [128, NBLK * len(pairs) * 2 * 128], F8,
                kind="ExternalInput").ap()
        if has_m:
            prm[f"mw{s}"] = nc.dram_tensor(
                f"mw{s}", [128, NBLK * NBLK * 128], F16,
                kind="ExternalInput").ap()
        prm[f"pw{s}"] = nc.dram_tensor(
            f"pw{s}", [128, NBLK * NBLK * 128], F16, kind="ExternalInput").ap()
        prm[f"tb{s}"] = nc.dram_tensor(f"tb{s}", [128, NBLK], F32,
                                       kind="ExternalInput").ap()
        prm[f"qb{s}"] = nc.dram_tensor(f"qb{s}", [128, NBLK], F32,
                                       kind="ExternalInput").ap()

    with tile.TileContext(nc) as tc:
        with tc.tile_pool(name="params", bufs=1) as ppool, \
             tc.tile_pool(name="x8", bufs=NSLOT) as xpool, \
             tc.tile_pool(name="x16", bufs=max(1, len(m_slots))) as mpool, \
             tc.tile_pool(name="a16", bufs=3) as apool, \
             tc.tile_pool(name="o16", bufs=4) as opool, \
             tc.tile_pool(name="psz", bufs=2, space="PSUM") as pszp, \
             tc.tile_pool(name="psw", bufs=2, space="PSUM") as pswp:

            pt = {}
            for name, ap in prm.items():
                t = ppool.tile(list(ap.shape), ap.dtype, tag=name, name=name)
                nc.sync.dma_start(t[:], ap)
                pt[name] = t

            for rep in range(repeat):
                # prefetch all slots' padded fp8 images (+fp16 for M slots)
                x8 = []
                x16 = {}
                for s in range(NSLOT):
                    t = xpool.tile([128, NBLK, PADHW], F8, tag="x8",
                                   name=f"x8{s}")
                    nc.sync.dma_start(
                        t[:], xin[s].rearrange("(a p) w -> p a w", a=NBLK,
                                               p=128))
                    x8.append(t)
                    if s in xm:
                        t16 = mpool.tile([128, NBLK, HW], F16, tag="x16",
                                         name=f"x16_{s}")
                        nc.sync.dma_start(
                            t16[:], xm[s].rearrange("(a p) w -> p a w",
                                                    a=NBLK, p=128))
                        x16[s] = t16

                a16 = {}

                def stage1(s):
                    tm, has_m = slot_modes[s]
                    offs, pairs = _offs1d(tm)
                    xt = x8[s][:]
                    xoff = xt.offset
                    at = apool.tile([128, NBLK, HW], F16, tag="a16",
                                    name=f"a16_{s}")
                    a16[s] = at
                    dk = pt.get(f"dk{s}")
                    if dk is not None:
                        dk3 = dk[:].rearrange("p (c i j m) -> p c i j m",
                                              c=NBLK, i=len(pairs), j=2, m=128)
                    if has_m:
                        mw3 = pt[f"mw{s}"][:].rearrange(
                            "p (c i m) -> p c i m", c=NBLK, i=NBLK, m=128)
                        xm16 = x16[s]
                    for cb in range(NBLK):
                        cbase = cb * PADHW
                        for hf4 in range(4):
                            psz = pszp.tile([128, 1024], F32, tag="psz",
                                            name="psz")
                            for q in range(2):
                                chunk = hf4 * 2 + q
                                wbase = (TPAD + chunk * 8) * R + PAD
                                pslice = psz[:, q * 512:(q + 1) * 512]
                                first = True
                                for i, (o1, delta, _) in enumerate(pairs):
                                    rhs = AP(xt.tensor,
                                             xoff + cbase + wbase + o1,
                                             [[NBLK * PADHW, 128], [delta, 2],
                                              [R, 8], [1, W]])
                                    nc.tensor.matmul(
                                        pslice, dk3[:, cb, i], rhs,
                                        start=first,
                                        stop=(not has_m and i == len(pairs) - 1),
                                        perf_mode=DRMODE)
                                    first = False
                                if has_m:
                                    off = chunk * 512
                                    for ib in range(NBLK):
                                        nc.tensor.matmul(
                                            pslice, mw3[:, cb, ib],
                                            xm16[:, ib, off:off + 512],
                                            start=first,
                                            stop=(ib == NBLK - 1))
                                        first = False
                            nc.scalar.activation(
                                at[:, cb, hf4 * 1024:(hf4 + 1) * 1024],
                                psz[:],
                                mybir.ActivationFunctionType.Gelu,
                                bias=pt[f"tb{s}"][:, cb:cb + 1], scale=1.0)

                def stage2(s):
                    at = a16[s]
                    pw3 = pt[f"pw{s}"][:].rearrange(
                        "p (c i m) -> p c i m", c=NBLK, i=NBLK, m=128)
                    for cb in range(NBLK):
                        o16 = opool.tile([128, HW], F16, tag="o16",
                                         name=f"o16_{s}_{cb}")
                        for quad in range(4):
                            psw = pswp.tile([128, 1024], F32, tag="psw",
                                            name="psw")
                            for q in range(2):
                                off = (quad * 2 + q) * 512
                                for ib in range(NBLK):
                                    nc.tensor.matmul(
                                        psw[:, q * 512:(q + 1) * 512],
                                        pw3[:, cb, ib],
                                        at[:, ib, off:off + 512],
                                        start=(ib == 0), stop=(ib == NBLK - 1))
                            dst = o16[:, quad * 1024:(quad + 1) * 1024]
                            qbcol = pt[f"qb{s}"][:, cb:cb + 1]
                            eng = copy_eng[s]
                            if eng == "act":
                                nc.scalar.activation(
                                    dst, psw[:],
                                    mybir.ActivationFunctionType.Identity,
                                    bias=qbcol, scale=1.0)
                            elif eng == "pool":
                                nc.gpsimd.tensor_scalar_add(dst, psw[:], qbcol)
                            else:
                                nc.vector.tensor_scalar_add(dst, psw[:], qbcol)
                        nc.sync.dma_start(
                            yout[s, cb * 128:(cb + 1) * 128, :], o16[:])

                # software pipeline: keep PE busy during gelu of prior slot
                stage1(0)
                stage1(1)
                stage2(0)
                stage1(2)
                stage2(1)
                stage1(3)
                stage2(2)
                stage2(3)
    nc.compile()
    return nc


def _plan(idx):
    """Assign elements to (core, slot); return slot_modes, elem[core][slot]."""
    by = [list(np.where(idx == t)[0]) for t in range(3)]
    groups = []  # (mode, has_m, [elems])
    for t, mode in ((0, "d1"), (1, "d2")):
        while len(by[t]) >= 8:
            groups.append([mode, False, by[t][:8]])
            by[t] = by[t][8:]
    for t, mode in ((0, "d1"), (1, "d2")):
        if by[t]:
            take = min(8 - len(by[t]), len(by[2]))
            g = by[t] + by[2][:take]
            by[2] = by[2][take:]
            by[t] = []
            groups.append([mode, take > 0, g])
    while by[2]:
        groups.append([None, True, by[2][:8]])
        by[2] = by[2][8:]
    while len(groups) > NSLOT:
        tapg = [g for g in groups if g[0] is not None]
        a, b = tapg[-2], tapg[-1]
        groups.remove(b)
        a[0] = "d12"
        a[1] = a[1] or b[1]
        a[2] += b[2]
        assert len(a[2]) <= 8
    for g in groups:
        g.append(len(g[2]))
        while len(g[2]) < 8:
            g[2].append(-1)
    while len(groups) < NSLOT:
        groups.append([None, False, [-1] * 8, 0])
    slot_modes = tuple((g[0], g[1]) for g in groups)
    elem = [[groups[s][2][c] for s in range(NSLOT)] for c in range(NCORES)]
    return slot_modes, elem


def _fold_params(kw):
    out = {}
    for i in range(3):
        g = kw[f"e{i}_g"]; b = kw[f"e{i}_b"]; m = kw[f"e{i}_m"]; v = kw[f"e{i}_v"]
        s = g / np.sqrt(v + EPS)
        t = b - m * s
        out[i] = dict(s=s.astype(np.float32), t=t.astype(np.float32),
                      pw=kw[f"e{i}_pw"].astype(np.float32),
                      pb=kw[f"e{i}_pb"].astype(np.float32))
        if i < 2:
            out[i]["k"] = (kw[f"e{i}_k"].reshape(C, 9) * s[:, None]).astype(np.float32)
        else:
            out[i]["M"] = (kw["e2_k"] * s[:, None]).astype(np.float32)
    return out


def _make_inmaps(x, idx, elem, slot_modes, fold):
    x8full = np.zeros((B, C, RH, R), NPF8)
    x8full[:, :, TPAD:TPAD + H, PAD:PAD + W] = x.astype(NPF8)
    x8full = x8full.reshape(B, C, PADHW)

    in_maps = []
    for c in range(NCORES):
        im = {}
        xs = np.zeros((NSLOT, C, PADHW), NPF8)
        for s in range(NSLOT):
            e = elem[c][s]
            if e >= 0:
                xs[s] = x8full[e]
        im["xin"] = xs
        for s, (tm, has_m) in enumerate(slot_modes):
            offs, pairs = _offs1d(tm)
            e = elem[c][s]
            t_e = idx[e] if e >= 0 else -1
            f = fold[t_e] if t_e >= 0 else None
            if offs:
                dk = np.zeros((128, NBLK, len(pairs), 2, 128), np.float32)
                if f is not None and t_e < 2:
                    dil = 1 if t_e == 0 else 2
                    myoffs = [dy * R + dx
                              for (dy, dx) in _offsets(dil)]
                    for i, (_, _, planes) in enumerate(pairs):
                        for j, o in enumerate(planes):
                            if o is not None and o in myoffs:
                                ki = myoffs.index(o)
                                for bk in range(NBLK):
                                    dk[np.arange(128), bk, i, j,
                                       np.arange(128)] = \
                                        f["k"][bk * 128:(bk + 1) * 128, ki]
                im[f"dk{s}"] = dk.astype(NPF8).reshape(128, -1)
            if has_m:
                mw = np.zeros((128, NBLK, NBLK, 128), np.float16)
                if f is not None and t_e == 2:
                    M = f["M"]
                    for cb in range(NBLK):
                        for ib in range(NBLK):
                            mw[:, cb, ib, :] = \
                                M[cb * 128:(cb + 1) * 128,
                                  ib * 128:(ib + 1) * 128].T.astype(np.float16)
                im[f"mw{s}"] = mw.reshape(128, -1)
                xm16 = np.zeros((C, HW), np.float16)
                if e >= 0:
                    xm16[:] = x[e].reshape(C, HW).astype(np.float16)
                im[f"xm{s}"] = xm16
            pw = np.zeros((128, NBLK, NBLK, 128), np.float16)
            tb = np.zeros((128, NBLK), np.float32)
            qb = np.zeros((128, NBLK), np.float32)
            if f is not None:
                P = f["pw"]
                for cb in range(NBLK):
                    for ib in range(NBLK):
                        pw[:, cb, ib, :] = \
                            P[cb * 128:(cb + 1) * 128,
                              ib * 128:(ib + 1) * 128].T.astype(np.float16)
                tb[:] = f["t"].reshape(NBLK, 128).T
                qb[:] = f["pb"].reshape(NBLK, 128).T
            im[f"pw{s}"] = pw.reshape(128, -1)
            im[f"tb{s}"] = tb
            im[f"qb{s}"] = qb
        in_maps.append(im)
    return in_maps


def _copy_eng(slot_modes):
    # GPSIMD/Pool cannot read PSUM, so the stage-2 copy goes to DVE or Act.
    return tuple(["dve", "dve", "dve", "act"][s] for s in range(NSLOT))


def kernel(**inputs):
    inputs = {k: np.asarray(v) for k, v in inputs.items()}
    x = np.ascontiguousarray(inputs["x"], np.float32)
    rw = np.asarray(inputs["rw"], np.float32)
    rb = np.asarray(inputs["rb"], np.float32)
    pooled = x.mean(axis=(2, 3), dtype=np.float32)
    logits = pooled @ rw.T + rb
    idx = logits.argmax(-1)

    slot_modes, elem = _plan(idx)
    fold = _fold_params(inputs)
    in_maps = _make_inmaps(x, idx, elem, slot_modes, fold)

    copy_eng = _copy_eng(slot_modes)
    key = (slot_modes, copy_eng)
    if key not in _CACHE:
        _CACHE[key] = build(slot_modes, copy_eng)
    nc = _CACHE[key]
    res = run_bass_kernel_spmd(nc, in_maps, core_ids=list(range(NCORES)),
                               trace=False)
    out = np.zeros((B, C, H, W), np.float32)
    for c in range(NCORES):
        yo = res.results[c]["yout"]
        for s in range(NSLOT):
            e = elem[c][s]
            if e >= 0:
                out[e] = yo[s].astype(np.float32).reshape(C, H, W)
    return out


# revision 22
# speedup vs baseline: 2.3104x; 1.2333x over previous
"""HeteroMoE layer for Trainium2, 8-core SPMD.

Top-1 routing with weight 1.0: out[b] = expert_{argmax(logits[b])}(x[b]).
Host computes routing and permutes the batch into 8 cores x 4 slots with a
uniform compile-time mode per slot. Depthwise 3x3 convs run as fp8e4m3
DoubleRow matmuls on the PE: each DR matmul applies TWO taps at once via a
diagonal-pair stationary operand and an overlapping-stride moving AP (the
k-subtile dim strides by the offset difference between the paired taps).
The expert-2 1x1 pre-matmul also runs as one fp8 DR matmul (K=256 via two
channel-block planes). BN is folded into the conv weights; gelu+BN-bias on
the scalar engine; the final pointwise 1x1 runs in fp16; PSUM->SBUF
copy+bias is spread across DVE/Pool/Act; fp16 output.
"""
import numpy as np
import ml_dtypes

import concourse.bacc as bacc
import concourse.tile as tile
import concourse.mybir as mybir
from concourse.ap import AP
from concourse.bass_utils import run_bass_kernel_spmd

F32 = mybir.dt.float32
F16 = mybir.dt.float16
F8 = mybir.dt.float8e4
NPF8 = ml_dtypes.float8_e4m3
DRMODE = mybir.MatmulPerfMode.DoubleRow

B, C, H, W = 32, 256, 64, 64
HW = H * W
NCORES = 8
NSLOT = B // NCORES
PAD = 2                  # left/right pad cols
TPAD = 2                 # top pad rows
BPAD = 3                 # bottom pad rows (+1 so the dummy-tap pair stays
                         # in bounds at the last chunk)
R = W + 2 * PAD          # padded row stride (68)
RH = H + TPAD + BPAD     # padded rows (69)
PADHW = R * RH           # 4692
NBLK = C // 128          # 2 channel blocks
EPS = 1e-5

_CACHE = {}


def _offsets(dil):
    return [(dy * dil, dx * dil) for dy in (-1, 0, 1) for dx in (-1, 0, 1)]


def _slot_offsets(mode):
    if mode == "d1":
        return _offsets(1)
    if mode == "d2":
        return _offsets(2)
    if mode == "d12":
        return _offsets(1) + [o for o in _offsets(2) if o != (0, 0)]
    return []


def _offs1d(mode):
    """Sorted 1-D offsets for the slot mode, plus per-pair layout.

    Returns (offs, pairs) where offs is the sorted offset list and pairs is a
    list of (o_first, delta, [plane0_off, plane1_off|None]); a None plane means
    zero weights (dummy tap reading in-bounds garbage at o_first+1).
    """
    offs = sorted(dy * R + dx for (dy, dx) in _slot_offsets(mode))
    pairs = []
    i = 0
    while i < len(offs):
        if i + 1 < len(offs):
            pairs.append((offs[i], offs[i + 1] - offs[i], (offs[i], offs[i + 1])))
            i += 2
        else:
            pairs.append((offs[i], 1, (offs[i], None)))
            i += 1
    return offs, pairs


def build(slot_modes, copy_eng, repeat=1):
    """slot_modes: (tapmode, has_m) per slot; copy_eng: per-slot engine for the
    stage-2 PSUM->SBUF copy ("dve"|"pool"|"act")."""
    nc = bacc.Bacc("TRN2", target_bir_lowering=False, debug=False,
                   num_devices=NCORES)
    xin = nc.dram_tensor("xin", [NSLOT, C, PADHW], F8, kind="ExternalInput").ap()
    m_slots = [s for s, (tm, hm) in enumerate(slot_modes) if hm]
    xm = {s: nc.dram_tensor(f"xm{s}", [C, HW], F16, kind="ExternalInput").ap()
          for s in m_slots}
    yout = nc.dram_tensor("yout", [NSLOT, C, HW], F16, kind="ExternalOutput").ap()
    prm = {}
    for s, (tm, has_m) in enumerate(slot_modes):
        offs, pairs = _offs1d(tm)
        if offs:
            prm[f"dk{s}"] = nc.dram_tensor(
                f"dk{s}", [128, NBLK * len(pairs) * 2 * 128], F8,
                kind="ExternalInput").ap()
        if has_m:
            prm[f"mw{s}"] = nc.dram_tensor(
                f"mw{s}", [128, NBLK * NBLK * 128], F16,
                kind="ExternalInput").ap()
        prm[f"pw{s}"] = nc.dram_tensor(
            f"pw{s}", [128, NBLK * NBLK * 128], F16 if has_m else F8,
            kind="ExternalInput").ap()
        prm[f"tb{s}"] = nc.dram_tensor(f"tb{s}", [128, NBLK], F32,
                                       kind="ExternalInput").ap()
        prm[f"qb{s}"] = nc.dram_tensor(f"qb{s}", [128, NBLK], F32,
                                       kind="ExternalInput").ap()

    with tile.TileContext(nc) as tc:
        with tc.tile_pool(name="params", bufs=1) as ppool, \
             tc.tile_pool(name="x8", bufs=NSLOT) as xpool, \
             tc.tile_pool(name="x16", bufs=max(1, len(m_slots))) as mpool, \
             tc.tile_pool(name="a16", bufs=3) as apool, \
             tc.tile_pool(name="o16", bufs=4) as opool, \
             tc.tile_pool(name="psz", bufs=2, space="PSUM") as pszp, \
             tc.tile_pool(name="psw", bufs=2, space="PSUM") as pswp:

            pt = {}

            def load_param(name):
                ap = prm[name]
                t = ppool.tile(list(ap.shape), ap.dtype, tag=name, name=name)
                nc.sync.dma_start(t[:], ap)
                pt[name] = t

            for rep in range(repeat):
                # DMA order: slot-0 data first so the PE starts ASAP, then the
                # rest slot by slot; stage-2 params last.
                x8 = []
                x16 = {}
                for s in range(NSLOT):
                    t = xpool.tile([128, NBLK, PADHW], F8, tag="x8",
                                   name=f"x8{s}")
                    nc.sync.dma_start(
                        t[:], xin[s].rearrange("(a p) w -> p a w", a=NBLK,
                                               p=128))
                    x8.append(t)
                    if rep == 0:
                        if f"dk{s}" in prm:
                            load_param(f"dk{s}")
                        load_param(f"tb{s}")
                    if s in xm:
                        t16 = mpool.tile([128, NBLK, HW], F16, tag="x16",
                                         name=f"x16_{s}")
                        nc.sync.dma_start(
                            t16[:], xm[s].rearrange("(a p) w -> p a w",
                                                    a=NBLK, p=128))
                        x16[s] = t16
                        if rep == 0:
                            load_param(f"mw{s}")
                if rep == 0:
                    for s in range(NSLOT):
                        load_param(f"pw{s}")
                        load_param(f"qb{s}")

                a16 = {}

                def stage1(s):
                    tm, has_m = slot_modes[s]
                    offs, pairs = _offs1d(tm)
                    xt = x8[s][:]
                    xoff = xt.offset
                    at = apool.tile([128, NBLK, HW], F16 if has_m else F8,
                                    tag="a16m" if has_m else "a8",
                                    name=f"a16_{s}")
                    a16[s] = at
                    dk = pt.get(f"dk{s}")
                    if dk is not None:
                        dk3 = dk[:].rearrange("p (c i j m) -> p c i j m",
                                              c=NBLK, i=len(pairs), j=2, m=128)
                    if has_m:
                        mw3 = pt[f"mw{s}"][:].rearrange(
                            "p (c i m) -> p c i m", c=NBLK, i=NBLK, m=128)
                        xm16 = x16[s]
                    for cb in range(NBLK):
                        cbase = cb * PADHW
                        for hf4 in range(4):
                            psz = pszp.tile([128, 1024], F32, tag="psz",
                                            name="psz")
                            for q in range(2):
                                chunk = hf4 * 2 + q
                                wbase = (TPAD + chunk * 8) * R + PAD
                                pslice = psz[:, q * 512:(q + 1) * 512]
                                first = True
                                for i, (o1, delta, _) in enumerate(pairs):
                                    rhs = AP(xt.tensor,
                                             xoff + cbase + wbase + o1,
                                             [[NBLK * PADHW, 128], [delta, 2],
                                              [R, 8], [1, W]])
                                    nc.tensor.matmul(
                                        pslice, dk3[:, cb, i], rhs,
                                        start=first,
                                        stop=(not has_m and i == len(pairs) - 1),
                                        perf_mode=DRMODE)
                                    first = False
                                if has_m:
                                    off = chunk * 512
                                    for ib in range(NBLK):
                                        nc.tensor.matmul(
                                            pslice, mw3[:, cb, ib],
                                            xm16[:, ib, off:off + 512],
                                            start=first,
                                            stop=(ib == NBLK - 1))
                                        first = False
                            nc.scalar.activation(
                                at[:, cb, hf4 * 1024:(hf4 + 1) * 1024],
                                psz[:],
                                mybir.ActivationFunctionType.Gelu,
                                bias=pt[f"tb{s}"][:, cb:cb + 1], scale=1.0)

                def stage2(s):
                    has_m = slot_modes[s][1]
                    at = a16[s][:]
                    aoff = at.offset
                    pw3 = pt[f"pw{s}"][:].rearrange(
                        "p (c i m) -> p c i m", c=NBLK, i=NBLK, m=128)
                    for cb in range(NBLK):
                        o16 = opool.tile([128, HW], F16, tag="o16",
                                         name=f"o16_{s}_{cb}")
                        for quad in range(4):
                            psw = pswp.tile([128, 1024], F32, tag="psw",
                                            name="psw")
                            for q in range(2):
                                off = (quad * 2 + q) * 512
                                if has_m:
                                    for ib in range(NBLK):
                                        nc.tensor.matmul(
                                            psw[:, q * 512:(q + 1) * 512],
                                            pw3[:, cb, ib],
                                            a16[s][:, ib, off:off + 512],
                                            start=(ib == 0),
                                            stop=(ib == NBLK - 1))
                                else:
                                    rhs = AP(at.tensor, aoff + off,
                                             [[NBLK * HW, 128], [HW, 2],
                                              [1, 512]])
                                    nc.tensor.matmul(
                                        psw[:, q * 512:(q + 1) * 512],
                                        pw3[:, cb], rhs,
                                        start=True, stop=True,
                                        perf_mode=DRMODE)
                            dst = o16[:, quad * 1024:(quad + 1) * 1024]
                            qbcol = pt[f"qb{s}"][:, cb:cb + 1]
                            if (quad + cb) % 2 == 0:
                                nc.vector.tensor_scalar_add(dst, psw[:], qbcol)
                            else:
                                nc.scalar.activation(
                                    dst, psw[:],
                                    mybir.ActivationFunctionType.Identity,
                                    bias=qbcol, scale=1.0)
                        for hh in range(2):
                            nc.sync.dma_start(
                                yout[s, cb * 128:(cb + 1) * 128,
                                     hh * 2048:(hh + 1) * 2048],
                                o16[:, hh * 2048:(hh + 1) * 2048])

                # software pipeline: keep PE busy during gelu of prior slot
                stage1(0)
                stage1(1)
                stage2(0)
                stage1(2)
                stage2(1)
                stage1(3)
                stage2(2)
                stage2(3)
    nc.compile()
    return nc


def _plan(idx):
    """Assign elements to (core, slot); return slot_modes, elem[core][slot]."""
    by = [list(np.where(idx == t)[0]) for t in range(3)]
    groups = []  # (mode, has_m, [elems])
    for t, mode in ((0, "d1"), (1, "d2")):
        while len(by[t]) >= 8:
            groups.append([mode, False, by[t][:8]])
            by[t] = by[t][8:]
    for t, mode in ((0, "d1"), (1, "d2")):
        if by[t]:
            take = min(8 - len(by[t]), len(by[2]))
            g = by[t] + by[2][:take]
            by[2] = by[2][take:]
            by[t] = []
            groups.append([mode, take > 0, g])
    while by[2]:
        groups.append([None, True, by[2][:8]])
        by[2] = by[2][8:]
    while len(groups) > NSLOT:
        tapg = [g for g in groups if g[0] is not None]
        a, b = tapg[-2], tapg[-1]
        groups.remove(b)
        a[0] = "d12"
        a[1] = a[1] or b[1]
        a[2] += b[2]
        assert len(a[2]) <= 8
    for g in groups:
        g.append(len(g[2]))
        while len(g[2]) < 8:
            g[2].append(-1)
    while len(groups) < NSLOT:
        groups.append([None, False, [-1] * 8, 0])
    slot_modes = tuple((g[0], g[1]) for g in groups)
    elem = [[groups[s][2][c] for s in range(NSLOT)] for c in range(NCORES)]
    return slot_modes, elem


def _fold_params(kw):
    out = {}
    for i in range(3):
        g = kw[f"e{i}_g"]; b = kw[f"e{i}_b"]; m = kw[f"e{i}_m"]; v = kw[f"e{i}_v"]
        s = g / np.sqrt(v + EPS)
        t = b - m * s
        out[i] = dict(s=s.astype(np.float32), t=t.astype(np.float32),
                      pw=kw[f"e{i}_pw"].astype(np.float32),
                      pb=kw[f"e{i}_pb"].astype(np.float32))
        if i < 2:
            out[i]["k"] = (kw[f"e{i}_k"].reshape(C, 9) * s[:, None]).astype(np.float32)
        else:
            out[i]["M"] = (kw["e2_k"] * s[:, None]).astype(np.float32)
    return out


def _make_inmaps(x, idx, elem, slot_modes, fold):
    x8full = np.zeros((B, C, RH, R), NPF8)
    x8full[:, :, TPAD:TPAD + H, PAD:PAD + W] = x.astype(NPF8)
    x8full = x8full.reshape(B, C, PADHW)

    in_maps = []
    for c in range(NCORES):
        im = {}
        xs = np.zeros((NSLOT, C, PADHW), NPF8)
        for s in range(NSLOT):
            e = elem[c][s]
            if e >= 0:
                xs[s] = x8full[e]
        im["xin"] = xs
        for s, (tm, has_m) in enumerate(slot_modes):
            offs, pairs = _offs1d(tm)
            e = elem[c][s]
            t_e = idx[e] if e >= 0 else -1
            f = fold[t_e] if t_e >= 0 else None
            if offs:
                dk = np.zeros((128, NBLK, len(pairs), 2, 128), np.float32)
                if f is not None and t_e < 2:
                    dil = 1 if t_e == 0 else 2
                    myoffs = [dy * R + dx
                              for (dy, dx) in _offsets(dil)]
                    for i, (_, _, planes) in enumerate(pairs):
                        for j, o in enumerate(planes):
                            if o is not None and o in myoffs:
                                ki = myoffs.index(o)
                                for bk in range(NBLK):
                                    dk[np.arange(128), bk, i, j,
                                       np.arange(128)] = \
                                        f["k"][bk * 128:(bk + 1) * 128, ki]
                im[f"dk{s}"] = dk.astype(NPF8).reshape(128, -1)
            if has_m:
                mw = np.zeros((128, NBLK, NBLK, 128), np.float16)
                if f is not None and t_e == 2:
                    M = f["M"]
                    for cb in range(NBLK):
                        for ib in range(NBLK):
                            mw[:, cb, ib, :] = \
                                M[cb * 128:(cb + 1) * 128,
                                  ib * 128:(ib + 1) * 128].T.astype(np.float16)
                im[f"mw{s}"] = mw.reshape(128, -1)
                xm16 = np.zeros((C, HW), np.float16)
                if e >= 0:
                    xm16[:] = x[e].reshape(C, HW).astype(np.float16)
                im[f"xm{s}"] = xm16
            pw = np.zeros((128, NBLK, NBLK, 128), np.float32)
            tb = np.zeros((128, NBLK), np.float32)
            qb = np.zeros((128, NBLK), np.float32)
            if f is not None:
                P = f["pw"]
                for cb in range(NBLK):
                    for ib in range(NBLK):
                        pw[:, cb, ib, :] = \
                            P[cb * 128:(cb + 1) * 128,
                              ib * 128:(ib + 1) * 128].T
                tb[:] = f["t"].reshape(NBLK, 128).T
                qb[:] = f["pb"].reshape(NBLK, 128).T
            im[f"pw{s}"] = pw.astype(
                np.float16 if has_m else NPF8).reshape(128, -1)
            im[f"tb{s}"] = tb
            im[f"qb{s}"] = qb
        in_maps.append(im)
    return in_maps


def _copy_eng(slot_modes):
    # GPSIMD/Pool cannot read PSUM, so the stage-2 copy goes to DVE or Act.
    return tuple(["dve", "dve", "dve", "act"][s] for s in range(NSLOT))


def kernel(**inputs):
    inputs = {k: np.asarray(v) for k, v in inputs.items()}
    x = np.ascontiguousarray(inputs["x"], np.float32)
    rw = np.asarray(inputs["rw"], np.float32)
    rb = np.asarray(inputs["rb"], np.float32)
    pooled = x.mean(axis=(2, 3), dtype=np.float32)
    logits = pooled @ rw.T + rb
    idx = logits.argmax(-1)

    slot_modes, elem = _plan(idx)
    fold = _fold_params(inputs)
    in_maps = _make_inmaps(x, idx, elem, slot_modes, fold)

    copy_eng = _copy_eng(slot_modes)
    key = (slot_modes, copy_eng)
    if key not in _CACHE:
        _CACHE[key] = build(slot_modes, copy_eng)
    nc = _CACHE[key]
    res = run_bass_kernel_spmd(nc, in_maps, core_ids=list(range(NCORES)),
                               trace=False)
    out = np.zeros((B, C, H, W), np.float32)
    for c in range(NCORES):
        yo = res.results[c]["yout"]
        for s in range(NSLOT):
            e = elem[c][s]
            if e >= 0:
                out[e] = yo[s].astype(np.float32).reshape(C, H, W)
    return out


# revision 26
# speedup vs baseline: 2.3884x; 1.0337x over previous
"""HeteroMoE layer for Trainium2, 8-core SPMD.

Top-1 routing with weight 1.0: out[b] = expert_{argmax(logits[b])}(x[b]).
Host computes routing and permutes the batch into 8 cores x 4 slots with a
uniform compile-time mode per slot. Depthwise 3x3 convs run as fp8e4m3
DoubleRow matmuls on the PE: each DR matmul applies TWO taps at once via a
diagonal-pair stationary operand and an overlapping-stride moving AP (the
k-subtile dim strides by the offset difference between the paired taps).
The expert-2 1x1 pre-matmul also runs as one fp8 DR matmul (K=256 via two
channel-block planes). BN is folded into the conv weights; gelu+BN-bias on
the scalar engine; the final pointwise 1x1 runs in fp16; PSUM->SBUF
copy+bias is spread across DVE/Pool/Act; fp16 output.
"""
import numpy as np
import ml_dtypes

import concourse.bacc as bacc
import concourse.tile as tile
import concourse.mybir as mybir
from concourse.ap import AP
from concourse.bass_utils import run_bass_kernel_spmd

F32 = mybir.dt.float32
F16 = mybir.dt.float16
F8 = mybir.dt.float8e4
NPF8 = ml_dtypes.float8_e4m3
DRMODE = mybir.MatmulPerfMode.DoubleRow

B, C, H, W = 32, 256, 64, 64
HW = H * W
NCORES = 8
NSLOT = B // NCORES
PAD = 2                  # left/right pad cols
TPAD = 2                 # top pad rows
BPAD = 3                 # bottom pad rows (+1 so the dummy-tap pair stays
                         # in bounds at the last chunk)
R = W + 2 * PAD          # padded row stride (68)
RH = H + TPAD + BPAD     # padded rows (69)
PADHW = R * RH           # 4692
NBLK = C // 128          # 2 channel blocks
EPS = 1e-5

_CACHE = {}


def _offsets(dil):
    return [(dy * dil, dx * dil) for dy in (-1, 0, 1) for dx in (-1, 0, 1)]


def _slot_offsets(mode):
    if mode == "d1":
        return _offsets(1)
    if mode == "d2":
        return _offsets(2)
    if mode == "d12":
        return _offsets(1) + [o for o in _offsets(2) if o != (0, 0)]
    return []


def _offs1d(mode):
    """Sorted 1-D offsets for the slot mode, plus per-pair layout.

    Returns (offs, pairs) where offs is the sorted offset list and pairs is a
    list of (o_first, delta, [plane0_off, plane1_off|None]); a None plane means
    zero weights (dummy tap reading in-bounds garbage at o_first+1).
    """
    offs = sorted(dy * R + dx for (dy, dx) in _slot_offsets(mode))
    pairs = []
    i = 0
    while i < len(offs):
        if i + 1 < len(offs):
            pairs.append((offs[i], offs[i + 1] - offs[i], (offs[i], offs[i + 1])))
            i += 2
        else:
            pairs.append((offs[i], 1, (offs[i], None)))
            i += 1
    return offs, pairs


def build(slot_modes, copy_eng, repeat=1):
    """slot_modes: (tapmode, has_m) per slot; copy_eng: per-slot engine for the
    stage-2 PSUM->SBUF copy ("dve"|"pool"|"act")."""
    nc = bacc.Bacc("TRN2", target_bir_lowering=False, debug=False,
                   num_devices=NCORES)
    xin = nc.dram_tensor("xin", [NSLOT, C, PADHW], F8, kind="ExternalInput").ap()
    m_slots = [s for s, (tm, hm) in enumerate(slot_modes) if hm]
    xm = {s: nc.dram_tensor(f"xm{s}", [C, HW], F16, kind="ExternalInput").ap()
          for s in m_slots}
    yout = nc.dram_tensor("yout", [NSLOT, C, HW], F16, kind="ExternalOutput").ap()
    prm = {}
    for s, (tm, has_m) in enumerate(slot_modes):
        offs, pairs = _offs1d(tm)
        if offs:
            prm[f"dk{s}"] = nc.dram_tensor(
                f"dk{s}", [128, NBLK * len(pairs) * 2 * 128], F8,
                kind="ExternalInput").ap()
        if has_m:
            prm[f"mw{s}"] = nc.dram_tensor(
                f"mw{s}", [128, NBLK * NBLK * 128], F16,
                kind="ExternalInput").ap()
        prm[f"pw{s}"] = nc.dram_tensor(
            f"pw{s}", [128, NBLK * NBLK * 128], F16 if has_m else F8,
            kind="ExternalInput").ap()
        prm[f"tb{s}"] = nc.dram_tensor(f"tb{s}", [128, NBLK], F32,
                                       kind="ExternalInput").ap()
        prm[f"qb{s}"] = nc.dram_tensor(f"qb{s}", [128, NBLK], F32,
                                       kind="ExternalInput").ap()

    with tile.TileContext(nc) as tc:
        with tc.tile_pool(name="params", bufs=1) as ppool, \
             tc.tile_pool(name="x8", bufs=NSLOT) as xpool, \
             tc.tile_pool(name="x16", bufs=max(1, len(m_slots))) as mpool, \
             tc.tile_pool(name="a16", bufs=3) as apool, \
             tc.tile_pool(name="o16", bufs=4) as opool, \
             tc.tile_pool(name="psz", bufs=2, space="PSUM") as pszp, \
             tc.tile_pool(name="psw", bufs=2, space="PSUM") as pswp:

            pt = {}

            def load_param(name):
                ap = prm[name]
                t = ppool.tile(list(ap.shape), ap.dtype, tag=name, name=name)
                nc.sync.dma_start(t[:], ap)
                pt[name] = t

            # PE warm-up: dummy matmul chain keeps the tensor engine busy
            # while the first DMAs land, so the p-state ramp completes before
            # real work starts.
            wtile = ppool.tile([128, 128], F8, tag="warm", name="warm")
            nc.gpsimd.memset(wtile[:], 0)
            wps = pszp.tile([128, 1024], F32, tag="psz", name="warmps")
            for _ in range(28):
                nc.tensor.matmul(wps[:, 0:128], wtile[:], wtile[:],
                                 start=True, stop=True)

            for rep in range(repeat):
                # DMA order: slot-0 data first so the PE starts ASAP, then the
                # rest slot by slot; stage-2 params last.
                x8 = []
                x16 = {}
                for s in range(NSLOT):
                    if rep == 0:
                        if f"dk{s}" in prm:
                            load_param(f"dk{s}")
                        load_param(f"tb{s}")
                    t = xpool.tile([128, NBLK, PADHW], F8, tag="x8",
                                   name=f"x8{s}")
                    src = xin[s].rearrange("(a p) w -> p a w", a=NBLK, p=128)
                    for bk in range(NBLK):
                        nc.sync.dma_start(t[:, bk], src[:, bk])
                    x8.append(t)
                    if s in xm:
                        t16 = mpool.tile([128, NBLK, HW], F16, tag="x16",
                                         name=f"x16_{s}")
                        nc.sync.dma_start(
                            t16[:], xm[s].rearrange("(a p) w -> p a w",
                                                    a=NBLK, p=128))
                        x16[s] = t16
                        if rep == 0:
                            load_param(f"mw{s}")
                if rep == 0:
                    for s in range(NSLOT):
                        load_param(f"pw{s}")
                        load_param(f"qb{s}")

                a16 = {}

                def stage1(s):
                    tm, has_m = slot_modes[s]
                    offs, pairs = _offs1d(tm)
                    xt = x8[s][:]
                    xoff = xt.offset
                    at = apool.tile([128, NBLK, HW], F16 if has_m else F8,
                                    tag="a16m" if has_m else "a8",
                                    name=f"a16_{s}")
                    a16[s] = at
                    dk = pt.get(f"dk{s}")
                    if dk is not None:
                        dk3 = dk[:].rearrange("p (c i j m) -> p c i j m",
                                              c=NBLK, i=len(pairs), j=2, m=128)
                    if has_m:
                        mw3 = pt[f"mw{s}"][:].rearrange(
                            "p (c i m) -> p c i m", c=NBLK, i=NBLK, m=128)
                        xm16 = x16[s]
                    for cb in range(NBLK):
                        cbase = cb * PADHW
                        for hf4 in range(4):
                            psz = pszp.tile([128, 1024], F32, tag="psz",
                                            name="psz")
                            for q in range(2):
                                chunk = hf4 * 2 + q
                                wbase = (TPAD + chunk * 8) * R + PAD
                                pslice = psz[:, q * 512:(q + 1) * 512]
                                first = True
                                for i, (o1, delta, _) in enumerate(pairs):
                                    rhs = AP(xt.tensor,
                                             xoff + cbase + wbase + o1,
                                             [[NBLK * PADHW, 128], [delta, 2],
                                              [R, 8], [1, W]])
                                    nc.tensor.matmul(
                                        pslice, dk3[:, cb, i], rhs,
                                        start=first,
                                        stop=(not has_m and i == len(pairs) - 1),
                                        perf_mode=DRMODE)
                                    first = False
                                if has_m:
                                    off = chunk * 512
                                    for ib in range(NBLK):
                                        nc.tensor.matmul(
                                            pslice, mw3[:, cb, ib],
                                            xm16[:, ib, off:off + 512],
                                            start=first,
                                            stop=(ib == NBLK - 1))
                                        first = False
                            nc.scalar.activation(
                                at[:, cb, hf4 * 1024:(hf4 + 1) * 1024],
                                psz[:],
                                mybir.ActivationFunctionType.Gelu,
                                bias=pt[f"tb{s}"][:, cb:cb + 1], scale=1.0)

                def stage2(s, wpool=None):
                    has_m = slot_modes[s][1]
                    at = a16[s][:]
                    aoff = at.offset
                    pw3 = pt[f"pw{s}"][:].rearrange(
                        "p (c i m) -> p c i m", c=NBLK, i=NBLK, m=128)
                    for cb in range(NBLK):
                        o16 = opool.tile([128, HW], F16, tag="o16",
                                         name=f"o16_{s}_{cb}")
                        for quad in range(4):
                            pp = wpool if wpool is not None else pswp
                            psw = pp.tile([128, 1024], F32,
                                          tag="psz" if wpool is not None
                                          else "psw",
                                          name="psw")
                            for q in range(2):
                                off = (quad * 2 + q) * 512
                                if has_m:
                                    for ib in range(NBLK):
                                        nc.tensor.matmul(
                                            psw[:, q * 512:(q + 1) * 512],
                                            pw3[:, cb, ib],
                                            a16[s][:, ib, off:off + 512],
                                            start=(ib == 0),
                                            stop=(ib == NBLK - 1))
                                else:
                                    rhs = AP(at.tensor, aoff + off,
                                             [[NBLK * HW, 128], [HW, 2],
                                              [1, 512]])
                                    nc.tensor.matmul(
                                        psw[:, q * 512:(q + 1) * 512],
                                        pw3[:, cb], rhs,
                                        start=True, stop=True,
                                        perf_mode=DRMODE)
                            dst = o16[:, quad * 1024:(quad + 1) * 1024]
                            qbcol = pt[f"qb{s}"][:, cb:cb + 1]
                            if (quad + cb) % 2 == 0:
                                nc.vector.tensor_scalar_add(dst, psw[:], qbcol)
                            else:
                                nc.scalar.activation(
                                    dst, psw[:],
                                    mybir.ActivationFunctionType.Identity,
                                    bias=qbcol, scale=1.0)
                        for hh in range(2):
                            nc.sync.dma_start(
                                yout[s, cb * 128:(cb + 1) * 128,
                                     hh * 2048:(hh + 1) * 2048],
                                o16[:, hh * 2048:(hh + 1) * 2048])

                # software pipeline: keep PE busy during gelu of prior slot;
                # finish with a cheap fp8-pointwise slot; the tail stage-2s
                # borrow the idle psz banks for deeper PSUM buffering
                stage1(0)
                stage1(1)
                stage2(0)
                stage1(2)
                stage1(3)
                stage2(2)
                stage2(3)
                stage2(1, wpool=pszp)
    nc.compile()
    return nc


def _plan(idx):
    """Assign elements to (core, slot); return slot_modes, elem[core][slot]."""
    by = [list(np.where(idx == t)[0]) for t in range(3)]
    groups = []  # (mode, has_m, [elems])
    for t, mode in ((0, "d1"), (1, "d2")):
        while len(by[t]) >= 8:
            groups.append([mode, False, by[t][:8]])
            by[t] = by[t][8:]
    for t, mode in ((0, "d1"), (1, "d2")):
        if by[t]:
            take = min(8 - len(by[t]), len(by[2]))
            g = by[t] + by[2][:take]
            by[2] = by[2][take:]
            by[t] = []
            groups.append([mode, take > 0, g])
    while by[2]:
        groups.append([None, True, by[2][:8]])
        by[2] = by[2][8:]
    while len(groups) > NSLOT:
        tapg = [g for g in groups if g[0] is not None]
        a, b = tapg[-2], tapg[-1]
        groups.remove(b)
        a[0] = "d12"
        a[1] = a[1] or b[1]
        a[2] += b[2]
        assert len(a[2]) <= 8
    for g in groups:
        g.append(len(g[2]))
        while len(g[2]) < 8:
            g[2].append(-1)
    while len(groups) < NSLOT:
        groups.append([None, False, [-1] * 8, 0])
    slot_modes = tuple((g[0], g[1]) for g in groups)
    elem = [[groups[s][2][c] for s in range(NSLOT)] for c in range(NCORES)]
    return slot_modes, elem


def _fold_params(kw):
    out = {}
    for i in range(3):
        g = kw[f"e{i}_g"]; b = kw[f"e{i}_b"]; m = kw[f"e{i}_m"]; v = kw[f"e{i}_v"]
        s = g / np.sqrt(v + EPS)
        t = b - m * s
        out[i] = dict(s=s.astype(np.float32), t=t.astype(np.float32),
                      pw=kw[f"e{i}_pw"].astype(np.float32),
                      pb=kw[f"e{i}_pb"].astype(np.float32))
        if i < 2:
            out[i]["k"] = (kw[f"e{i}_k"].reshape(C, 9) * s[:, None]).astype(np.float32)
        else:
            out[i]["M"] = (kw["e2_k"] * s[:, None]).astype(np.float32)
    return out


def _make_inmaps(x, idx, elem, slot_modes, fold):
    x8full = np.zeros((B, C, RH, R), NPF8)
    x8full[:, :, TPAD:TPAD + H, PAD:PAD + W] = x.astype(NPF8)
    x8full = x8full.reshape(B, C, PADHW)

    in_maps = []
    for c in range(NCORES):
        im = {}
        xs = np.zeros((NSLOT, C, PADHW), NPF8)
        for s in range(NSLOT):
            e = elem[c][s]
            if e >= 0:
                xs[s] = x8full[e]
        im["xin"] = xs
        for s, (tm, has_m) in enumerate(slot_modes):
            offs, pairs = _offs1d(tm)
            e = elem[c][s]
            t_e = idx[e] if e >= 0 else -1
            f = fold[t_e] if t_e >= 0 else None
            if offs:
                dk = np.zeros((128, NBLK, len(pairs), 2, 128), np.float32)
                if f is not None and t_e < 2:
                    dil = 1 if t_e == 0 else 2
                    myoffs = [dy * R + dx
                              for (dy, dx) in _offsets(dil)]
                    for i, (_, _, planes) in enumerate(pairs):
                        for j, o in enumerate(planes):
                            if o is not None and o in myoffs:
                                ki = myoffs.index(o)
                                for bk in range(NBLK):
                                    dk[np.arange(128), bk, i, j,
                                       np.arange(128)] = \
                                        f["k"][bk * 128:(bk + 1) * 128, ki]
                im[f"dk{s}"] = dk.astype(NPF8).reshape(128, -1)
            if has_m:
                mw = np.zeros((128, NBLK, NBLK, 128), np.float16)
                if f is not None and t_e == 2:
                    M = f["M"]
                    for cb in range(NBLK):
                        for ib in range(NBLK):
                            mw[:, cb, ib, :] = \
                                M[cb * 128:(cb + 1) * 128,
                                  ib * 128:(ib + 1) * 128].T.astype(np.float16)
                im[f"mw{s}"] = mw.reshape(128, -1)
                xm16 = np.zeros((C, HW), np.float16)
                if e >= 0:
                    xm16[:] = x[e].reshape(C, HW).astype(np.float16)
                im[f"xm{s}"] = xm16
            pw = np.zeros((128, NBLK, NBLK, 128), np.float32)
            tb = np.zeros((128, NBLK), np.float32)
            qb = np.zeros((128, NBLK), np.float32)
            if f is not None:
                P = f["pw"]
                for cb in range(NBLK):
                    for ib in range(NBLK):
                        pw[:, cb, ib, :] = \
                            P[cb * 128:(cb + 1) * 128,
                              ib * 128:(ib + 1) * 128].T
                tb[:] = f["t"].reshape(NBLK, 128).T
                qb[:] = f["pb"].reshape(NBLK, 128).T
            im[f"pw{s}"] = pw.astype(
                np.float16 if has_m else NPF8).reshape(128, -1)
            im[f"tb{s}"] = tb
            im[f"qb{s}"] = qb
        in_maps.append(im)
    return in_maps


def _copy_eng(slot_modes):
    # GPSIMD/Pool cannot read PSUM, so the stage-2 copy goes to DVE or Act.
    return tuple(["dve", "dve", "dve", "act"][s] for s in range(NSLOT))


def kernel(**inputs):
    inputs = {k: np.asarray(v) for k, v in inputs.items()}
    x = np.ascontiguousarray(inputs["x"], np.float32)
    rw = np.asarray(inputs["rw"], np.float32)
    rb = np.asarray(inputs["rb"], np.float32)
    pooled = x.mean(axis=(2, 3), dtype=np.float32)
    logits = pooled @ rw.T + rb
    idx = logits.argmax(-1)

    slot_modes, elem = _plan(idx)
    fold = _fold_params(inputs)
    in_maps = _make_inmaps(x, idx, elem, slot_modes, fold)

    copy_eng = _copy_eng(slot_modes)
    key = (slot_modes, copy_eng)
    if key not in _CACHE:
        _CACHE[key] = build(slot_modes, copy_eng)
    nc = _CACHE[key]
    res = run_bass_kernel_spmd(nc, in_maps, core_ids=list(range(NCORES)),
                               trace=False)
    out = np.zeros((B, C, H, W), np.float32)
    for c in range(NCORES):
        yo = res.results[c]["yout"]
        for s in range(NSLOT):
            e = elem[c][s]
            if e >= 0:
                out[e] = yo[s].astype(np.float32).reshape(C, H, W)
    return out


# revision 30
# speedup vs baseline: 2.5738x; 1.0776x over previous
"""HeteroMoE layer for Trainium2, 8-core SPMD.

Top-1 routing with weight 1.0: out[b] = expert_{argmax(logits[b])}(x[b]).
Host computes routing and permutes the batch into 8 cores x 4 slots with a
uniform compile-time mode per slot. Depthwise 3x3 convs run as fp8e4m3
DoubleRow matmuls on the PE: each DR matmul applies TWO taps at once via a
diagonal-pair stationary operand and an overlapping-stride moving AP (the
k-subtile dim strides by the offset difference between the paired taps).
The expert-2 1x1 pre-matmul also runs as one fp8 DR matmul (K=256 via two
channel-block planes). BN is folded into the conv weights; gelu+BN-bias on
the scalar engine; the final pointwise 1x1 runs in fp16; PSUM->SBUF
copy+bias is spread across DVE/Pool/Act; fp16 output.
"""
import numpy as np
import ml_dtypes

import concourse.bacc as bacc
import concourse.tile as tile
import concourse.mybir as mybir
from concourse.ap import AP
from concourse.bass_utils import run_bass_kernel_spmd

F32 = mybir.dt.float32
F16 = mybir.dt.float16
F8 = mybir.dt.float8e4
NPF8 = ml_dtypes.float8_e4m3
DRMODE = mybir.MatmulPerfMode.DoubleRow

B, C, H, W = 32, 256, 64, 64
HW = H * W
NCORES = 8
NSLOT = B // NCORES
PAD = 2                  # left/right pad cols
TPAD = 2                 # top pad rows
BPAD = 3                 # bottom pad rows (+1 so the dummy-tap pair stays
                         # in bounds at the last chunk)
R = W + 2 * PAD          # padded row stride (68)
RH = H + TPAD + BPAD     # padded rows (69)
PADHW = R * RH           # 4692
NBLK = C // 128          # 2 channel blocks
EPS = 1e-5

_CACHE = {}


def _offsets(dil):
    return [(dy * dil, dx * dil) for dy in (-1, 0, 1) for dx in (-1, 0, 1)]


def _slot_offsets(mode):
    if mode == "d1":
        return _offsets(1)
    if mode == "d2":
        return _offsets(2)
    if mode == "d12":
        return _offsets(1) + [o for o in _offsets(2) if o != (0, 0)]
    return []


def _offs1d(mode):
    """Sorted 1-D offsets for the slot mode, plus per-pair layout.

    Returns (offs, pairs) where offs is the sorted offset list and pairs is a
    list of (o_first, delta, [plane0_off, plane1_off|None]); a None plane means
    zero weights (dummy tap reading in-bounds garbage at o_first+1).
    """
    offs = sorted(dy * R + dx for (dy, dx) in _slot_offsets(mode))
    pairs = []
    i = 0
    while i < len(offs):
        if i + 1 < len(offs):
            pairs.append((offs[i], offs[i + 1] - offs[i], (offs[i], offs[i + 1])))
            i += 2
        else:
            pairs.append((offs[i], 1, (offs[i], None)))
            i += 1
    return offs, pairs


def build(slot_modes, copy_eng, repeat=1):
    """slot_modes: (tapmode, has_m) per slot; copy_eng: per-slot engine for the
    stage-2 PSUM->SBUF copy ("dve"|"pool"|"act")."""
    nc = bacc.Bacc("TRN2", target_bir_lowering=False, debug=False,
                   num_devices=NCORES)
    xin = nc.dram_tensor("xin", [NSLOT, C, PADHW], F8, kind="ExternalInput").ap()
    m_slots = [s for s, (tm, hm) in enumerate(slot_modes) if hm]
    xm = {s: nc.dram_tensor(f"xm{s}", [C, HW], F16, kind="ExternalInput").ap()
          for s in m_slots}
    yout = nc.dram_tensor("yout", [NSLOT, C, HW], F16, kind="ExternalOutput").ap()
    prm = {}
    for s, (tm, has_m) in enumerate(slot_modes):
        offs, pairs = _offs1d(tm)
        if offs:
            prm[f"dk{s}"] = nc.dram_tensor(
                f"dk{s}", [128, NBLK * len(pairs) * 2 * 128], F8,
                kind="ExternalInput").ap()
        if has_m:
            prm[f"mw{s}"] = nc.dram_tensor(
                f"mw{s}", [128, NBLK * NBLK * 128], F16,
                kind="ExternalInput").ap()
        prm[f"pw{s}"] = nc.dram_tensor(
            f"pw{s}", [128, NBLK * NBLK * 128], F16 if has_m else F8,
            kind="ExternalInput").ap()
        prm[f"tb{s}"] = nc.dram_tensor(f"tb{s}", [128, NBLK], F32,
                                       kind="ExternalInput").ap()
        prm[f"qb{s}"] = nc.dram_tensor(f"qb{s}", [128, NBLK], F32,
                                       kind="ExternalInput").ap()

    with tile.TileContext(nc) as tc:
        with tc.tile_pool(name="params", bufs=1) as ppool, \
             tc.tile_pool(name="x8", bufs=NSLOT) as xpool, \
             tc.tile_pool(name="x16", bufs=max(1, len(m_slots))) as mpool, \
             tc.tile_pool(name="a16", bufs=3) as apool, \
             tc.tile_pool(name="o16", bufs=4) as opool, \
             tc.tile_pool(name="psz", bufs=2, space="PSUM") as pszp, \
             tc.tile_pool(name="psw", bufs=2, space="PSUM") as pswp:

            pt = {}

            def load_param(name):
                ap = prm[name]
                t = ppool.tile(list(ap.shape), ap.dtype, tag=name, name=name)
                nc.sync.dma_start(t[:], ap)
                pt[name] = t

            # PE warm-up: dummy matmul chain keeps the tensor engine busy
            # while the first DMAs land, so the p-state ramp completes before
            # real work starts.
            wtile = ppool.tile([128, 128], F8, tag="warm", name="warm")
            nc.gpsimd.memset(wtile[:], 0)
            wps = pszp.tile([128, 1024], F32, tag="psz", name="warmps")
            for _ in range(30):
                nc.tensor.matmul(wps[:, 0:128], wtile[:], wtile[:],
                                 start=True, stop=True)

            for rep in range(repeat):
                # DMA order: slot-0 data first so the PE starts ASAP, then the
                # rest slot by slot; stage-2 params last.
                x8 = []
                x16 = {}
                for s in range(NSLOT):
                    t = xpool.tile([128, NBLK, PADHW], F8, tag="x8",
                                   name=f"x8{s}")
                    src = xin[s].rearrange("(a p) w -> p a w", a=NBLK, p=128)
                    nc.sync.dma_start(t[:, 0], src[:, 0])
                    if rep == 0:
                        if f"dk{s}" in prm:
                            load_param(f"dk{s}")
                        load_param(f"tb{s}")
                    nc.sync.dma_start(t[:, 1], src[:, 1])
                    x8.append(t)
                    if s in xm:
                        t16 = mpool.tile([128, NBLK, HW], F16, tag="x16",
                                         name=f"x16_{s}")
                        nc.sync.dma_start(
                            t16[:], xm[s].rearrange("(a p) w -> p a w",
                                                    a=NBLK, p=128))
                        x16[s] = t16
                        if rep == 0:
                            load_param(f"mw{s}")
                if rep == 0:
                    for s in range(NSLOT):
                        load_param(f"pw{s}")
                        load_param(f"qb{s}")

                a16 = {}

                def stage1(s):
                    tm, has_m = slot_modes[s]
                    offs, pairs = _offs1d(tm)
                    xt = x8[s][:]
                    xoff = xt.offset
                    at = apool.tile([128, NBLK, HW], F16 if has_m else F8,
                                    tag="a16m" if has_m else "a8",
                                    name=f"a16_{s}")
                    a16[s] = at
                    dk = pt.get(f"dk{s}")
                    if dk is not None:
                        dk3 = dk[:].rearrange("p (c i j m) -> p c i j m",
                                              c=NBLK, i=len(pairs), j=2, m=128)
                    if has_m:
                        mw3 = pt[f"mw{s}"][:].rearrange(
                            "p (c i m) -> p c i m", c=NBLK, i=NBLK, m=128)
                        xm16 = x16[s]
                    for cb in range(NBLK):
                        cbase = cb * PADHW
                        for hf4 in range(4):
                            psz = pszp.tile([128, 1024], F32, tag="psz",
                                            name="psz")
                            for q in range(2):
                                chunk = hf4 * 2 + q
                                wbase = (TPAD + chunk * 8) * R + PAD
                                pslice = psz[:, q * 512:(q + 1) * 512]
                                first = True
                                for i, (o1, delta, _) in enumerate(pairs):
                                    rhs = AP(xt.tensor,
                                             xoff + cbase + wbase + o1,
                                             [[NBLK * PADHW, 128], [delta, 2],
                                              [R, 8], [1, W]])
                                    nc.tensor.matmul(
                                        pslice, dk3[:, cb, i], rhs,
                                        start=first,
                                        stop=(not has_m and i == len(pairs) - 1),
                                        perf_mode=DRMODE)
                                    first = False
                                if has_m:
                                    off = chunk * 512
                                    for ib in range(NBLK):
                                        nc.tensor.matmul(
                                            pslice, mw3[:, cb, ib],
                                            xm16[:, ib, off:off + 512],
                                            start=first,
                                            stop=(ib == NBLK - 1))
                                        first = False
                            nc.scalar.activation(
                                at[:, cb, hf4 * 1024:(hf4 + 1) * 1024],
                                psz[:],
                                mybir.ActivationFunctionType.Gelu,
                                bias=pt[f"tb{s}"][:, cb:cb + 1], scale=1.0)

                def stage2(s, wpool=None, fine_out=False):
                    has_m = slot_modes[s][1]
                    at = a16[s][:]
                    aoff = at.offset
                    pw3 = pt[f"pw{s}"][:].rearrange(
                        "p (c i m) -> p c i m", c=NBLK, i=NBLK, m=128)
                    for cb in range(NBLK):
                        o16 = opool.tile([128, HW], F16, tag="o16",
                                         name=f"o16_{s}_{cb}")
                        for quad in range(4):
                            pp = wpool if wpool is not None else pswp
                            psw = pp.tile([128, 1024], F32,
                                          tag="psz" if wpool is not None
                                          else "psw",
                                          name="psw")
                            for q in range(2):
                                off = (quad * 2 + q) * 512
                                if has_m:
                                    for ib in range(NBLK):
                                        nc.tensor.matmul(
                                            psw[:, q * 512:(q + 1) * 512],
                                            pw3[:, cb, ib],
                                            a16[s][:, ib, off:off + 512],
                                            start=(ib == 0),
                                            stop=(ib == NBLK - 1))
                                else:
                                    rhs = AP(at.tensor, aoff + off,
                                             [[NBLK * HW, 128], [HW, 2],
                                              [1, 512]])
                                    nc.tensor.matmul(
                                        psw[:, q * 512:(q + 1) * 512],
                                        pw3[:, cb], rhs,
                                        start=True, stop=True,
                                        perf_mode=DRMODE)
                            dst = o16[:, quad * 1024:(quad + 1) * 1024]
                            qbcol = pt[f"qb{s}"][:, cb:cb + 1]
                            if (quad + cb) % 2 == 0:
                                nc.vector.tensor_scalar_add(dst, psw[:], qbcol)
                            else:
                                nc.scalar.activation(
                                    dst, psw[:],
                                    mybir.ActivationFunctionType.Identity,
                                    bias=qbcol, scale=1.0)
                        nout = 4 if fine_out else 2
                        for hh in range(nout):
                            w0 = HW // nout
                            nc.sync.dma_start(
                                yout[s, cb * 128:(cb + 1) * 128,
                                     hh * w0:(hh + 1) * w0],
                                o16[:, hh * w0:(hh + 1) * w0])

                # software pipeline: keep PE busy during gelu of prior slot;
                # the tail stage-2 borrows the idle psz banks for deeper
                # PSUM buffering and streams output at finer grain
                stage1(0)
                stage1(1)
                stage2(0)
                stage1(2)
                stage2(1)
                stage1(3)
                stage2(2, fine_out=True)
                stage2(3, wpool=pszp, fine_out=True)
    nc.compile()
    return nc


def _plan(idx):
    """Assign elements to (core, slot); return slot_modes, elem[core][slot]."""
    by = [list(np.where(idx == t)[0]) for t in range(3)]
    groups = []  # (mode, has_m, [elems])
    for t, mode in ((0, "d1"), (1, "d2")):
        while len(by[t]) >= 8:
            groups.append([mode, False, by[t][:8]])
            by[t] = by[t][8:]
    for t, mode in ((0, "d1"), (1, "d2")):
        if by[t]:
            take = min(8 - len(by[t]), len(by[2]))
            g = by[t] + by[2][:take]
            by[2] = by[2][take:]
            by[t] = []
            groups.append([mode, take > 0, g])
    while by[2]:
        groups.append([None, True, by[2][:8]])
        by[2] = by[2][8:]
    while len(groups) > NSLOT:
        tapg = [g for g in groups if g[0] is not None]
        a, b = tapg[-2], tapg[-1]
        groups.remove(b)
        a[0] = "d12"
        a[1] = a[1] or b[1]
        a[2] += b[2]
        assert len(a[2]) <= 8
    for g in groups:
        g.append(len(g[2]))
        while len(g[2]) < 8:
            g[2].append(-1)
    while len(groups) < NSLOT:
        groups.append([None, False, [-1] * 8, 0])
    slot_modes = tuple((g[0], g[1]) for g in groups)
    elem = [[groups[s][2][c] for s in range(NSLOT)] for c in range(NCORES)]
    return slot_modes, elem


def _fold_params(kw):
    out = {}
    for i in range(3):
        g = kw[f"e{i}_g"]; b = kw[f"e{i}_b"]; m = kw[f"e{i}_m"]; v = kw[f"e{i}_v"]
        s = g / np.sqrt(v + EPS)
        t = b - m * s
        out[i] = dict(s=s.astype(np.float32), t=t.astype(np.float32),
                      pw=kw[f"e{i}_pw"].astype(np.float32),
                      pb=kw[f"e{i}_pb"].astype(np.float32))
        if i < 2:
            out[i]["k"] = (kw[f"e{i}_k"].reshape(C, 9) * s[:, None]).astype(np.float32)
        else:
            out[i]["M"] = (kw["e2_k"] * s[:, None]).astype(np.float32)
    return out


def _make_inmaps(x, idx, elem, slot_modes, fold):
    x8full = np.zeros((B, C, RH, R), NPF8)
    x8full[:, :, TPAD:TPAD + H, PAD:PAD + W] = x.astype(NPF8)
    x8full = x8full.reshape(B, C, PADHW)

    in_maps = []
    for c in range(NCORES):
        im = {}
        xs = np.zeros((NSLOT, C, PADHW), NPF8)
        for s in range(NSLOT):
            e = elem[c][s]
            if e >= 0:
                xs[s] = x8full[e]
        im["xin"] = xs
        for s, (tm, has_m) in enumerate(slot_modes):
            offs, pairs = _offs1d(tm)
            e = elem[c][s]
            t_e = idx[e] if e >= 0 else -1
            f = fold[t_e] if t_e >= 0 else None
            if offs:
                dk = np.zeros((128, NBLK, len(pairs), 2, 128), np.float32)
                if f is not None and t_e < 2:
                    dil = 1 if t_e == 0 else 2
                    myoffs = [dy * R + dx
                              for (dy, dx) in _offsets(dil)]
                    for i, (_, _, planes) in enumerate(pairs):
                        for j, o in enumerate(planes):
                            if o is not None and o in myoffs:
                                ki = myoffs.index(o)
                                for bk in range(NBLK):
                                    dk[np.arange(128), bk, i, j,
                                       np.arange(128)] = \
                                        f["k"][bk * 128:(bk + 1) * 128, ki]
                im[f"dk{s}"] = dk.astype(NPF8).reshape(128, -1)
            if has_m:
                mw = np.zeros((128, NBLK, NBLK, 128), np.float16)
                if f is not None and t_e == 2:
                    M = f["M"]
                    for cb in range(NBLK):
                        for ib in range(NBLK):
                            mw[:, cb, ib, :] = \
                                M[cb * 128:(cb + 1) * 128,
                                  ib * 128:(ib + 1) * 128].T.astype(np.float16)
                im[f"mw{s}"] = mw.reshape(128, -1)
                xm16 = np.zeros((C, HW), np.float16)
                if e >= 0:
                    xm16[:] = x[e].reshape(C, HW).astype(np.float16)
                im[f"xm{s}"] = xm16
            pw = np.zeros((128, NBLK, NBLK, 128), np.float32)
            tb = np.zeros((128, NBLK), np.float32)
            qb = np.zeros((128, NBLK), np.float32)
            if f is not None:
                P = f["pw"]
                for cb in range(NBLK):
                    for ib in range(NBLK):
                        pw[:, cb, ib, :] = \
                            P[cb * 128:(cb + 1) * 128,
                              ib * 128:(ib + 1) * 128].T
                tb[:] = f["t"].reshape(NBLK, 128).T
                qb[:] = f["pb"].reshape(NBLK, 128).T
            im[f"pw{s}"] = pw.astype(
                np.float16 if has_m else NPF8).reshape(128, -1)
            im[f"tb{s}"] = tb
            im[f"qb{s}"] = qb
        in_maps.append(im)
    return in_maps


def _copy_eng(slot_modes):
    # GPSIMD/Pool cannot read PSUM, so the stage-2 copy goes to DVE or Act.
    return tuple(["dve", "dve", "dve", "act"][s] for s in range(NSLOT))


def kernel(**inputs):
    inputs = {k: np.asarray(v) for k, v in inputs.items()}
    x = np.ascontiguousarray(inputs["x"], np.float32)
    rw = np.asarray(inputs["rw"], np.float32)
    rb = np.asarray(inputs["rb"], np.float32)
    pooled = x.mean(axis=(2, 3), dtype=np.float32)
    logits = pooled @ rw.T + rb
    idx = logits.argmax(-1)

    slot_modes, elem = _plan(idx)
    fold = _fold_params(inputs)
    in_maps = _make_inmaps(x, idx, elem, slot_modes, fold)

    copy_eng = _copy_eng(slot_modes)
    key = (slot_modes, copy_eng)
    if key not in _CACHE:
        _CACHE[key] = build(slot_modes, copy_eng)
    nc = _CACHE[key]
    res = run_bass_kernel_spmd(nc, in_maps, core_ids=list(range(NCORES)),
                               trace=False)
    out = np.zeros((B, C, H, W), np.float32)
    for c in range(NCORES):
        yo = res.results[c]["yout"]
        for s in range(NSLOT):
            e = elem[c][s]
            if e >= 0:
                out[e] = yo[s].astype(np.float32).reshape(C, H, W)
    return out


# revision 34
# speedup vs baseline: 2.6384x; 1.0251x over previous
"""HeteroMoE layer for Trainium2, 8-core SPMD.

Top-1 routing with weight 1.0: out[b] = expert_{argmax(logits[b])}(x[b]).
Host computes routing and permutes the batch into 8 cores x 4 slots with a
uniform compile-time mode per slot. Depthwise 3x3 convs run as fp8e4m3
DoubleRow matmuls on the PE: each DR matmul applies TWO taps at once via a
diagonal-pair stationary operand and an overlapping-stride moving AP (the
k-subtile dim strides by the offset difference between the paired taps).
The expert-2 1x1 pre-matmul also runs as one fp8 DR matmul (K=256 via two
channel-block planes). BN is folded into the conv weights; gelu+BN-bias on
the scalar engine; the final pointwise 1x1 runs in fp16; PSUM->SBUF
copy+bias is spread across DVE/Pool/Act; fp16 output.
"""
import numpy as np
import ml_dtypes

import concourse.bacc as bacc
import concourse.tile as tile
import concourse.mybir as mybir
from concourse.ap import AP
from concourse.bass_utils import run_bass_kernel_spmd

F32 = mybir.dt.float32
F16 = mybir.dt.float16
F8 = mybir.dt.float8e4
NPF8 = ml_dtypes.float8_e4m3
DRMODE = mybir.MatmulPerfMode.DoubleRow

B, C, H, W = 32, 256, 64, 64
HW = H * W
NCORES = 8
NSLOT = B // NCORES
PAD = 2                  # left/right pad cols
TPAD = 2                 # top pad rows
BPAD = 3                 # bottom pad rows (+1 so the dummy-tap pair stays
                         # in bounds at the last chunk)
R = W + 2 * PAD          # padded row stride (68)
RH = H + TPAD + BPAD     # padded rows (69)
PADHW = R * RH           # 4692
NBLK = C // 128          # 2 channel blocks
EPS = 1e-5

_CACHE = {}


def _offsets(dil):
    return [(dy * dil, dx * dil) for dy in (-1, 0, 1) for dx in (-1, 0, 1)]


def _slot_offsets(mode):
    if mode == "d1":
        return _offsets(1)
    if mode == "d2":
        return _offsets(2)
    if mode == "d12":
        return _offsets(1) + [o for o in _offsets(2) if o != (0, 0)]
    return []


def _offs1d(mode):
    """Sorted 1-D offsets for the slot mode, plus per-pair layout.

    Returns (offs, pairs) where offs is the sorted offset list and pairs is a
    list of (o_first, delta, [plane0_off, plane1_off|None]); a None plane means
    zero weights (dummy tap reading in-bounds garbage at o_first+1).
    """
    offs = sorted(dy * R + dx for (dy, dx) in _slot_offsets(mode))
    pairs = []
    i = 0
    while i < len(offs):
        if i + 1 < len(offs):
            pairs.append((offs[i], offs[i + 1] - offs[i], (offs[i], offs[i + 1])))
            i += 2
        else:
            pairs.append((offs[i], 1, (offs[i], None)))
            i += 1
    return offs, pairs


def build(slot_modes, copy_eng, repeat=1):
    """slot_modes: (tapmode, has_m) per slot; copy_eng: per-slot engine for the
    stage-2 PSUM->SBUF copy ("dve"|"pool"|"act")."""
    nc = bacc.Bacc("TRN2", target_bir_lowering=False, debug=False,
                   num_devices=NCORES)
    xin = nc.dram_tensor("xin", [NSLOT, C, PADHW], F8, kind="ExternalInput").ap()
    m_slots = [s for s, (tm, hm) in enumerate(slot_modes) if hm]
    xm = {s: nc.dram_tensor(f"xm{s}", [C, HW], F16, kind="ExternalInput").ap()
          for s in m_slots}
    yout = nc.dram_tensor("yout", [NSLOT, C, HW], F16, kind="ExternalOutput").ap()
    prm = {}
    for s, (tm, has_m) in enumerate(slot_modes):
        offs, pairs = _offs1d(tm)
        if offs:
            prm[f"dk{s}"] = nc.dram_tensor(
                f"dk{s}", [128, NBLK * len(pairs) * 2 * 128], F8,
                kind="ExternalInput").ap()
        if has_m:
            prm[f"mw{s}"] = nc.dram_tensor(
                f"mw{s}", [128, NBLK * NBLK * 128], F16,
                kind="ExternalInput").ap()
        prm[f"pw{s}"] = nc.dram_tensor(
            f"pw{s}", [128, NBLK * NBLK * 128], F16 if has_m else F8,
            kind="ExternalInput").ap()
        prm[f"tb{s}"] = nc.dram_tensor(f"tb{s}", [128, NBLK], F32,
                                       kind="ExternalInput").ap()
        prm[f"qb{s}"] = nc.dram_tensor(f"qb{s}", [128, NBLK], F32,
                                       kind="ExternalInput").ap()

    with tile.TileContext(nc) as tc:
        with tc.tile_pool(name="params", bufs=1) as ppool, \
             tc.tile_pool(name="x8", bufs=NSLOT) as xpool, \
             tc.tile_pool(name="x16", bufs=max(1, len(m_slots))) as mpool, \
             tc.tile_pool(name="a16", bufs=3) as apool, \
             tc.tile_pool(name="o16", bufs=4) as opool, \
             tc.tile_pool(name="psz", bufs=2, space="PSUM") as pszp, \
             tc.tile_pool(name="psw", bufs=2, space="PSUM") as pswp:

            pt = {}

            def load_param(name):
                ap = prm[name]
                t = ppool.tile(list(ap.shape), ap.dtype, tag=name, name=name)
                nc.sync.dma_start(t[:], ap)
                pt[name] = t

            # PE warm-up: dummy matmul chain keeps the tensor engine busy
            # while the first DMAs land, so the p-state ramp completes before
            # real work starts.
            wtile = ppool.tile([128, 128], F8, tag="warm", name="warm")
            nc.gpsimd.memset(wtile[:], 0)
            wps = pszp.tile([128, 1024], F32, tag="psz", name="warmps")
            for _ in range(30):
                nc.tensor.matmul(wps[:, 0:128], wtile[:], wtile[:],
                                 start=True, stop=True)

            for rep in range(repeat):
                # DMA order: slot-0 data first so the PE starts ASAP, then the
                # rest slot by slot; stage-2 params last.
                x8 = []
                x16 = {}
                for s in range(NSLOT):
                    t = xpool.tile([128, NBLK, PADHW], F8, tag="x8",
                                   name=f"x8{s}")
                    src = xin[s].rearrange("(a p) w -> p a w", a=NBLK, p=128)
                    if s == 0:
                        # fine-grained first upload: top rows + first dk half
                        # land ASAP so the PE starts early
                        htop = (TPAD + 2 * 8) * R
                        nc.sync.dma_start(t[:, 0, 0:htop], src[:, 0, 0:htop])
                        if rep == 0 and f"dk{s}" in prm:
                            load_param(f"dk{s}")
                        if rep == 0:
                            load_param(f"tb{s}")
                        nc.sync.dma_start(t[:, 0, htop:PADHW],
                                          src[:, 0, htop:PADHW])
                        nc.sync.dma_start(t[:, 1], src[:, 1])
                    else:
                        if rep == 0:
                            if f"dk{s}" in prm:
                                load_param(f"dk{s}")
                            load_param(f"tb{s}")
                        nc.sync.dma_start(t[:, 0], src[:, 0])
                        nc.sync.dma_start(t[:, 1], src[:, 1])
                    x8.append(t)
                    if s in xm:
                        t16 = mpool.tile([128, NBLK, HW], F16, tag="x16",
                                         name=f"x16_{s}")
                        nc.sync.dma_start(
                            t16[:], xm[s].rearrange("(a p) w -> p a w",
                                                    a=NBLK, p=128))
                        x16[s] = t16
                        if rep == 0:
                            load_param(f"mw{s}")
                if rep == 0:
                    for s in range(NSLOT):
                        load_param(f"pw{s}")
                        load_param(f"qb{s}")

                a16 = {}

                def stage1(s):
                    tm, has_m = slot_modes[s]
                    offs, pairs = _offs1d(tm)
                    xt = x8[s][:]
                    xoff = xt.offset
                    at = apool.tile([128, NBLK, HW], F16 if has_m else F8,
                                    tag="a16m" if has_m else "a8",
                                    name=f"a16_{s}")
                    a16[s] = at
                    dk = pt.get(f"dk{s}")
                    if dk is not None:
                        dk3 = dk[:].rearrange("p (c i j m) -> p c i j m",
                                              c=NBLK, i=len(pairs), j=2, m=128)
                    if has_m:
                        mw3 = pt[f"mw{s}"][:].rearrange(
                            "p (c i m) -> p c i m", c=NBLK, i=NBLK, m=128)
                        xm16 = x16[s]
                    for cb in range(NBLK):
                        cbase = cb * PADHW
                        for hf4 in range(4):
                            psz = pszp.tile([128, 1024], F32, tag="psz",
                                            name="psz")
                            for q in range(2):
                                chunk = hf4 * 2 + q
                                wbase = (TPAD + chunk * 8) * R + PAD
                                pslice = psz[:, q * 512:(q + 1) * 512]
                                first = True
                                for i, (o1, delta, _) in enumerate(pairs):
                                    rhs = AP(xt.tensor,
                                             xoff + cbase + wbase + o1,
                                             [[NBLK * PADHW, 128], [delta, 2],
                                              [R, 8], [1, W]])
                                    nc.tensor.matmul(
                                        pslice, dk3[:, cb, i], rhs,
                                        start=first,
                                        stop=(not has_m and i == len(pairs) - 1),
                                        perf_mode=DRMODE)
                                    first = False
                                if has_m:
                                    off = chunk * 512
                                    for ib in range(NBLK):
                                        nc.tensor.matmul(
                                            pslice, mw3[:, cb, ib],
                                            xm16[:, ib, off:off + 512],
                                            start=first,
                                            stop=(ib == NBLK - 1))
                                        first = False
                            nc.scalar.activation(
                                at[:, cb, hf4 * 1024:(hf4 + 1) * 1024],
                                psz[:],
                                mybir.ActivationFunctionType.Gelu,
                                bias=pt[f"tb{s}"][:, cb:cb + 1], scale=1.0)

                def stage2(s, alt_pool=False, fine_out=False, tail=False):
                    has_m = slot_modes[s][1]
                    at = a16[s][:]
                    aoff = at.offset
                    pw3 = pt[f"pw{s}"][:].rearrange(
                        "p (c i m) -> p c i m", c=NBLK, i=NBLK, m=128)
                    for cb in range(NBLK):
                        o16 = opool.tile([128, HW], F16, tag="o16",
                                         name=f"o16_{s}_{cb}")
                        for quad in range(4):
                            if alt_pool and (quad + cb) % 2 == 1:
                                psw = pszp.tile([128, 1024], F32, tag="psz",
                                                name="psw")
                            else:
                                psw = pswp.tile([128, 1024], F32, tag="psw",
                                                name="psw")
                            for q in range(2):
                                off = (quad * 2 + q) * 512
                                if has_m:
                                    for ib in range(NBLK):
                                        nc.tensor.matmul(
                                            psw[:, q * 512:(q + 1) * 512],
                                            pw3[:, cb, ib],
                                            a16[s][:, ib, off:off + 512],
                                            start=(ib == 0),
                                            stop=(ib == NBLK - 1))
                                else:
                                    rhs = AP(at.tensor, aoff + off,
                                             [[NBLK * HW, 128], [HW, 2],
                                              [1, 512]])
                                    nc.tensor.matmul(
                                        psw[:, q * 512:(q + 1) * 512],
                                        pw3[:, cb], rhs,
                                        start=True, stop=True,
                                        perf_mode=DRMODE)
                            dst = o16[:, quad * 1024:(quad + 1) * 1024]
                            qbcol = pt[f"qb{s}"][:, cb:cb + 1]
                            if tail:
                                on_dve = (quad + cb) % 2 == 0
                            else:
                                on_dve = quad != 2  # 3 of 4 on DVE: Act paces
                            if on_dve:
                                nc.vector.tensor_scalar_add(dst, psw[:], qbcol)
                            else:
                                nc.scalar.activation(
                                    dst, psw[:],
                                    mybir.ActivationFunctionType.Identity,
                                    bias=qbcol, scale=1.0)
                        nout = 4 if fine_out else 2
                        for hh in range(nout):
                            w0 = HW // nout
                            nc.sync.dma_start(
                                yout[s, cb * 128:(cb + 1) * 128,
                                     hh * w0:(hh + 1) * w0],
                                o16[:, hh * w0:(hh + 1) * w0])

                # software pipeline: keep PE busy during gelu of prior slot;
                # the tail stage-2 borrows the idle psz banks for deeper
                # PSUM buffering and streams output at finer grain
                stage1(0)
                stage1(1)
                stage2(0)
                stage1(2)
                stage2(1)
                stage1(3)
                stage2(2, fine_out=True)
                stage2(3, alt_pool=True, fine_out=True, tail=True)
    nc.compile()
    return nc


def _plan(idx):
    """Assign elements to (core, slot); return slot_modes, elem[core][slot]."""
    by = [list(np.where(idx == t)[0]) for t in range(3)]
    groups = []  # (mode, has_m, [elems])
    for t, mode in ((0, "d1"), (1, "d2")):
        while len(by[t]) >= 8:
            groups.append([mode, False, by[t][:8]])
            by[t] = by[t][8:]
    for t, mode in ((0, "d1"), (1, "d2")):
        if by[t]:
            take = min(8 - len(by[t]), len(by[2]))
            g = by[t] + by[2][:take]
            by[2] = by[2][take:]
            by[t] = []
            groups.append([mode, take > 0, g])
    while by[2]:
        groups.append([None, True, by[2][:8]])
        by[2] = by[2][8:]
    while len(groups) > NSLOT:
        tapg = [g for g in groups if g[0] is not None]
        a, b = tapg[-2], tapg[-1]
        groups.remove(b)
        a[0] = "d12"
        a[1] = a[1] or b[1]
        a[2] += b[2]
        assert len(a[2]) <= 8
    for g in groups:
        g.append(len(g[2]))
        while len(g[2]) < 8:
            g[2].append(-1)
    while len(groups) < NSLOT:
        groups.append([None, False, [-1] * 8, 0])
    slot_modes = tuple((g[0], g[1]) for g in groups)
    elem = [[groups[s][2][c] for s in range(NSLOT)] for c in range(NCORES)]
    return slot_modes, elem


def _fold_params(kw):
    out = {}
    for i in range(3):
        g = kw[f"e{i}_g"]; b = kw[f"e{i}_b"]; m = kw[f"e{i}_m"]; v = kw[f"e{i}_v"]
        s = g / np.sqrt(v + EPS)
        t = b - m * s
        out[i] = dict(s=s.astype(np.float32), t=t.astype(np.float32),
                      pw=kw[f"e{i}_pw"].astype(np.float32),
                      pb=kw[f"e{i}_pb"].astype(np.float32))
        if i < 2:
            out[i]["k"] = (kw[f"e{i}_k"].reshape(C, 9) * s[:, None]).astype(np.float32)
        else:
            out[i]["M"] = (kw["e2_k"] * s[:, None]).astype(np.float32)
    return out


def _make_inmaps(x, idx, elem, slot_modes, fold):
    x8full = np.zeros((B, C, RH, R), NPF8)
    x8full[:, :, TPAD:TPAD + H, PAD:PAD + W] = x.astype(NPF8)
    x8full = x8full.reshape(B, C, PADHW)

    in_maps = []
    for c in range(NCORES):
        im = {}
        xs = np.zeros((NSLOT, C, PADHW), NPF8)
        for s in range(NSLOT):
            e = elem[c][s]
            if e >= 0:
                xs[s] = x8full[e]
        im["xin"] = xs
        for s, (tm, has_m) in enumerate(slot_modes):
            offs, pairs = _offs1d(tm)
            e = elem[c][s]
            t_e = idx[e] if e >= 0 else -1
            f = fold[t_e] if t_e >= 0 else None
            if offs:
                dk = np.zeros((128, NBLK, len(pairs), 2, 128), np.float32)
                if f is not None and t_e < 2:
                    dil = 1 if t_e == 0 else 2
                    myoffs = [dy * R + dx
                              for (dy, dx) in _offsets(dil)]
                    for i, (_, _, planes) in enumerate(pairs):
                        for j, o in enumerate(planes):
                            if o is not None and o in myoffs:
                                ki = myoffs.index(o)
                                for bk in range(NBLK):
                                    dk[np.arange(128), bk, i, j,
                                       np.arange(128)] = \
                                        f["k"][bk * 128:(bk + 1) * 128, ki]
                im[f"dk{s}"] = dk.astype(NPF8).reshape(128, -1)
            if has_m:
                mw = np.zeros((128, NBLK, NBLK, 128), np.float16)
                if f is not None and t_e == 2:
                    M = f["M"]
                    for cb in range(NBLK):
                        for ib in range(NBLK):
                            mw[:, cb, ib, :] = \
                                M[cb * 128:(cb + 1) * 128,
                                  ib * 128:(ib + 1) * 128].T.astype(np.float16)
                im[f"mw{s}"] = mw.reshape(128, -1)
                xm16 = np.zeros((C, HW), np.float16)
                if e >= 0:
                    xm16[:] = x[e].reshape(C, HW).astype(np.float16)
                im[f"xm{s}"] = xm16
            pw = np.zeros((128, NBLK, NBLK, 128), np.float32)
            tb = np.zeros((128, NBLK), np.float32)
            qb = np.zeros((128, NBLK), np.float32)
            if f is not None:
                P = f["pw"]
                for cb in range(NBLK):
                    for ib in range(NBLK):
                        pw[:, cb, ib, :] = \
                            P[cb * 128:(cb + 1) * 128,
                              ib * 128:(ib + 1) * 128].T
                tb[:] = f["t"].reshape(NBLK, 128).T
                qb[:] = f["pb"].reshape(NBLK, 128).T
            im[f"pw{s}"] = pw.astype(
                np.float16 if has_m else NPF8).reshape(128, -1)
            im[f"tb{s}"] = tb
            im[f"qb{s}"] = qb
        in_maps.append(im)
    return in_maps


def _copy_eng(slot_modes):
    # GPSIMD/Pool cannot read PSUM, so the stage-2 copy goes to DVE or Act.
    return tuple(["dve", "dve", "dve", "act"][s] for s in range(NSLOT))


def kernel(**inputs):
    inputs = {k: np.asarray(v) for k, v in inputs.items()}
    x = np.ascontiguousarray(inputs["x"], np.float32)
    rw = np.asarray(inputs["rw"], np.float32)
    rb = np.asarray(inputs["rb"], np.float32)
    pooled = x.mean(axis=(2, 3), dtype=np.float32)
    logits = pooled @ rw.T + rb
    idx = logits.argmax(-1)

    slot_modes, elem = _plan(idx)
    fold = _fold_params(inputs)
    in_maps = _make_inmaps(x, idx, elem, slot_modes, fold)

    copy_eng = _copy_eng(slot_modes)
    key = (slot_modes, copy_eng)
    if key not in _CACHE:
        _CACHE[key] = build(slot_modes, copy_eng)
    nc = _CACHE[key]
    res = run_bass_kernel_spmd(nc, in_maps, core_ids=list(range(NCORES)),
                               trace=False)
    out = np.zeros((B, C, H, W), np.float32)
    for c in range(NCORES):
        yo = res.results[c]["yout"]
        for s in range(NSLOT):
            e = elem[c][s]
            if e >= 0:
                out[e] = yo[s].astype(np.float32).reshape(C, H, W)
    return out
